# revision 6
# baseline (speedup 1.0000x reference)
"""GAT (2-layer, 8-head) + graph pooling + MLP on 8 TRN2 NeuronCores.

Single merged SPMD program (one dispatch per call) with on-device collectives:
  - layer-1 gather table is REPLICATED: every core computes all 50176 rows
    from the full x (cheap [32,520] matmuls) -> no AllGather-1
  - layer-2: AllGather only the bf16 h1^T [64, 6272] per core (0.8MB), then
    every core computes the full layer-2 table locally
  - tables use f-major head layout (row = [f0h0..f0h7, f1h0..]) so the
    per-edge msg multiply qualifies for DVE 2x_1P mode
  - per layer: edge pass with dma_gather by src (640-wide bf16 rows),
    attention ex = exp(leakyrelu(as+ad)), one-hot S matmuls for
    segment-softmax sum/den per 128-dst block
  - layer-2 epilogue pools nodes into 3 local 128-graph windows, windows
    land in a [2048, 512] global graph grid via dma_gather, then a
    ReduceScatter hands core c the pooled rows [256c, 256c+256)
  - each core runs the small MLP on its 256 graphs -> out [1, 256]

`_build_merged(meta, iters=K)` unrolls the whole kernel K times inside one
NEFF (same buffers) so test.py can subtract the host-dispatch RTT:
t_exec = (wall(K) - wall(1)) / (K - 1).
"""
import os
import time
import numpy as np
import ml_dtypes
from contextlib import ExitStack

os.environ.setdefault("JAX_PLATFORMS", "")  # allow axon platform auto-detect

LAST_TIMES = {}

import concourse.bass as bass
import concourse.bacc as bacc
import concourse.mybir as mybir
import concourse.tile as tile
from concourse.masks import make_identity

BF = ml_dtypes.bfloat16

N = 50000
E = 800000
NF = 32
H = 8
F = 64
HF = 512
G = 2000
NEG = 0.2
NCORES = 8
NSH = N // NCORES            # 6250 dst nodes per core
NBLK = (NSH + 127) // 128    # 49 blocks
NSHP = NBLK * 128            # 6272
NPADN = NCORES * NSHP        # 50176 padded-global rows (core-major)
HALF = 4 * NSHP              # 25088 = cores 0-3 -> table A (int16 idx range)
NWIN = 3                     # 128-graph windows per core (local range < 384)
WROWS = 512                  # winbuf rows: 384 window rows + zero pad
ZROW = 400                   # a guaranteed-zero winbuf row for out-of-range
GRID = 2048                  # global graph grid rows (>= G, 16 tiles of 128)
GSH = GRID // NCORES         # 256 graphs per core after ReduceScatter

F32 = mybir.dt.float32
BF16 = mybir.dt.bfloat16
I16 = mybir.dt.int16

_cache = {}


# ---------------------------------------------------------------- host prep
def _preprocess(edge_index, batch):
    src = np.concatenate([edge_index[0], np.arange(N, dtype=np.int64)]).astype(np.int64)
    dst = np.concatenate([edge_index[1], np.arange(N, dtype=np.int64)]).astype(np.int64)
    core = dst // NSH
    dloc = (dst - core * NSH).astype(np.int64)
    blk = dloc // 128
    # padded-global table row of the source node (core-major, stride NSHP)
    spad = (src // NSH) * NSHP + (src % NSH)
    tab = (spad >= HALF).astype(np.int64)

    # group key per edge: (core, blk, tab)
    key = (core * NBLK + blk) * 2 + tab
    order = np.argsort(key, kind="stable")
    src_s, dst_s, key_s = spad[order], dst[order], key[order]
    counts = np.bincount(key_s, minlength=NCORES * NBLK * 2).reshape(NCORES, NBLK * 2)

    # uniform chunk counts across cores
    K = np.ceil(counts.max(axis=0) / 128.0).astype(np.int64)  # [NBLK*2]
    TOTCH = int(K.sum())
    choff = np.concatenate([[0], np.cumsum(K)])  # chunk offset per group

    # per-core flat edge slot arrays [TOTCH*128]
    srci = np.zeros((NCORES, TOTCH * 128), np.int16)
    dsti = np.zeros((NCORES, TOTCH * 128), np.int16)
    dstl = np.full((NCORES, TOTCH * 128), -1.0, np.float32)

    gstart = np.concatenate([[0], np.cumsum(counts.reshape(-1))[:-1]])
    gs = gstart.reshape(NCORES, NBLK * 2)
    for c in range(NCORES):
        for g in range(NBLK * 2):
            n = counts[c, g]
            if n == 0:
                continue
            s0 = gs[c, g]
            es, ed = src_s[s0:s0 + n], dst_s[s0:s0 + n]
            o0 = choff[g] * 128
            t = g & 1
            srci[c, o0:o0 + n] = (es - t * HALF).astype(np.int16)
            dl = (ed - c * NSH).astype(np.int64)
            dsti[c, o0:o0 + n] = dl.astype(np.int16)
            dstl[c, o0:o0 + n] = (dl - (g // 2) * 128).astype(np.float32)

    # gather runs: per group, runs of <=8 chunks
    gathers = []  # (tab, chunk0, nch)
    for g in range(NBLK * 2):
        k = int(K[g])
        c0 = int(choff[g])
        while k > 0:
            nch = min(k, 8)
            gathers.append((g & 1, c0, nch))
            c0 += nch
            k -= nch

    def wrap16(v):  # [n] -> [128, n//16] column-major wrap, replicated
        n = v.shape[0]
        return np.tile(v.reshape(n // 16, 16).T, (8, 1)).astype(np.int16)

    idx_src = [np.concatenate(
        [wrap16(srci[c, c0 * 128:(c0 + nch) * 128]) for (_, c0, nch) in gathers], axis=1)
        for c in range(NCORES)]
    idx_dst = [np.concatenate(
        [wrap16(dsti[c, c0 * 128:(c0 + nch) * 128]) for (_, c0, nch) in gathers], axis=1)
        for c in range(NCORES)]
    dstl_t = [dstl[c].reshape(TOTCH, 128).T.copy() for c in range(NCORES)]

    # block boundaries in chunk space: block b covers chunks [choff[2b], choff[2b+2])
    blk_first = [int(choff[2 * b]) for b in range(NBLK)]
    blk_last = [int(choff[2 * b + 2]) - 1 for b in range(NBLK)]

    # graph-local window values per (lane, block, window)
    g0 = [int(batch[c * NSH]) for c in range(NCORES)]
    glw = []
    for c in range(NCORES):
        gmax = int(batch[(c + 1) * NSH - 1]) if c < NCORES - 1 else int(batch[N - 1])
        assert gmax - g0[c] < NWIN * 128, "graph-local id range exceeds windows"
        a = np.full((128, NBLK * NWIN), -1.0, np.float32)
        for b in range(NBLK):
            nn = min(128, NSH - b * 128)
            nodes = c * NSH + b * 128 + np.arange(nn)
            gl = batch[nodes] - g0[c]
            for w in range(NWIN):
                a[:nn, b * NWIN + w] = gl - 128 * w
        glw.append(a)

    # grid gather indices: grid row r <- winbuf row (r - g0) if in window
    # range else the zeroed row ZROW
    gridx = []
    for c in range(NCORES):
        r = np.arange(GRID, dtype=np.int64) - g0[c]
        idx = np.where((r >= 0) & (r < NWIN * 128), r, ZROW).astype(np.int16)
        gridx.append(np.concatenate(
            [np.tile(idx[t * 128:(t + 1) * 128].reshape(8, 16).T, (8, 1))
             for t in range(GRID // 128)], axis=1))

    return dict(TOTCH=TOTCH, gathers=gathers, blk_first=blk_first, blk_last=blk_last,
                idx_src=idx_src, idx_dst=idx_dst, dstl=dstl_t, glw=glw, g0=g0,
                gridx=gridx)


def _wcat(Wmat, a_vec):
    # [fin, H*F] weight + per-head attention vec -> [fin, H] alpha weight
    fin = Wmat.shape[0]
    Wr = Wmat.reshape(fin, H, F)
    return np.einsum("fhk,hk->fh", Wr, a_vec)


def _fmajor(Wmat):
    # [fin, H*F] h-major columns -> f-major columns [fin, F*H]
    fin = Wmat.shape[0]
    return Wmat.reshape(fin, H, F).transpose(0, 2, 1).reshape(fin, H * F)


# ------------------------------------------------------------- device build
def _edge_pass(nc, tc, ctx, meta, tabA, tabB, adtab, tagb, consts, epil):
    """Shared edge-processing pass. epil(b, num_ps, den_ps) emits the block
    epilogue after the block's last chunk. f-major head layout throughout."""
    IC_off = 0
    sb = ctx.enter_context(tc.tile_pool(name=f"eg{tagb}", bufs=3))
    sbm = ctx.enter_context(tc.tile_pool(name=f"em{tagb}", bufs=6))
    psN = ctx.enter_context(tc.tile_pool(name=f"pn{tagb}", bufs=2, space="PSUM"))
    psD = ctx.enter_context(tc.tile_pool(name=f"pd{tagb}", bufs=2, space="PSUM"))

    iota_bf = consts["iota_bf"]
    dstl_sb = consts["dstl_sb"]
    isrc_sb = consts["isrc_sb"]
    idst_sb = consts["idst_sb"]

    num_ps = den_ps = None
    cur_blk = -1
    for (t, c0, nch) in meta["gathers"]:
        n = nch * 128
        cols = nch * 8
        gt = sb.tile([128, nch, 640], BF16, tag="maing")
        nc.gpsimd.dma_gather(
            out_ap=gt[:], in_ap=(tabA if t == 0 else tabB),
            idxs_ap=isrc_sb[:, IC_off:IC_off + cols],
            num_idxs=n, num_idxs_reg=n, elem_size=640)
        adt = sb.tile([128, nch, 128], BF16, tag="adg")
        nc.gpsimd.dma_gather(
            out_ap=adt[:], in_ap=adtab[:],
            idxs_ap=idst_sb[:, IC_off:IC_off + cols],
            num_idxs=n, num_idxs_reg=n, elem_size=128)
        IC_off += cols

        e_st = sb.tile([128, nch, 8], F32, tag="est")
        nc.vector.tensor_tensor(
            out=e_st[:], in0=gt[:, :, 512:520], in1=adt[:, :, 0:8],
            op=mybir.AluOpType.add)
        e_fl = e_st[:].rearrange("p a b -> p (a b)")
        t_sc = sb.tile([128, nch * 8], F32, tag="esc")
        nc.vector.tensor_scalar(out=t_sc[:], in0=e_fl, scalar1=NEG, scalar2=None,
                                op0=mybir.AluOpType.mult)
        nc.vector.tensor_tensor(out=e_fl, in0=e_fl, in1=t_sc[:],
                                op=mybir.AluOpType.max)
        ex_st = sb.tile([128, nch * 8], BF16, tag="exs")
        nc.scalar.activation(ex_st[:], e_fl, mybir.ActivationFunctionType.Exp)

        for j in range(nch):
            ch = c0 + j
            if num_ps is None or ch > meta["blk_last"][cur_blk]:
                cur_blk += 1
                num_ps = psN.tile([128, 512], F32, tag="nps")
                den_ps = psD.tile([128, 8], F32, tag="dps")
            S = sbm.tile([128, 128], BF16, tag="S")
            nc.vector.tensor_scalar(
                out=S[:], in0=iota_bf[:], scalar1=dstl_sb[:, ch:ch + 1],
                scalar2=None, op0=mybir.AluOpType.is_equal)
            msg = sbm.tile([128, 512], BF16, tag="msg")
            nc.vector.tensor_tensor(
                out=msg[:].rearrange("p (h f) -> p h f", h=H),
                in0=gt[:, j, 0:512].rearrange("p (h f) -> p h f", h=H),
                in1=ex_st[:, 8 * j:8 * j + 8].unsqueeze(2).to_broadcast([128, H, F]),
                op=mybir.AluOpType.mult)
            first = ch == meta["blk_first"][cur_blk]
            last = ch == meta["blk_last"][cur_blk]
            nc.tensor.matmul(num_ps[:], lhsT=S[:], rhs=msg[:], start=first, stop=last)
            nc.tensor.matmul(den_ps[:], lhsT=S[:], rhs=ex_st[:, 8 * j:8 * j + 8],
                             start=first, stop=last)
            if last:
                epil(cur_blk, num_ps, den_ps)


def _load_edge_consts(nc, tc, ctx, meta, inp, tagb):
    consts = {}
    cp = ctx.enter_context(tc.tile_pool(name=f"ec{tagb}", bufs=1))
    IC = sum(nch * 8 for (_, _, nch) in meta["gathers"])
    isrc_sb = cp.tile([128, IC], I16)
    nc.sync.dma_start(isrc_sb[:], inp["idx_src"][:])
    idst_sb = cp.tile([128, IC], I16)
    nc.sync.dma_start(idst_sb[:], inp["idx_dst"][:])
    dstl_sb = cp.tile([128, meta["TOTCH"]], F32)
    nc.sync.dma_start(dstl_sb[:], inp["dstl"][:])
    iota_bf = cp.tile([128, 128], BF16)
    nc.sync.dma_start(iota_bf[:], inp["iota_bf"][:])
    consts.update(isrc_sb=isrc_sb, idst_sb=idst_sb, dstl_sb=dstl_sb, iota_bf=iota_bf)
    return consts


def _emit_iter(nc, tc, ctx, meta, io, groups, it):
    """Emit one full kernel iteration. io: dict of dram tensors."""
    tg = f"_{it}"
    cp = ctx.enter_context(tc.tile_pool(name=f"wc{tg}", bufs=1))
    w1_sb = cp.tile([32, 520], BF16)
    nc.sync.dma_start(w1_sb[:], io["i_w1"][:])
    wad1_sb = cp.tile([32, 8], BF16)
    nc.sync.dma_start(wad1_sb[:], io["i_wad1"][:])
    b1_sb = cp.tile([128, 64], F32)
    nc.sync.dma_start(b1_sb[:], io["i_b1"][:])
    w2_sb = cp.tile([64, 520], BF16)
    nc.sync.dma_start(w2_sb[:], io["i_w2"][:])
    wad2_sb = cp.tile([64, 8], BF16)
    nc.sync.dma_start(wad2_sb[:], io["i_wad2"][:])
    b2_sb = cp.tile([128, 512], F32)
    nc.sync.dma_start(b2_sb[:], io["i_b2"][:])
    glw_sb = cp.tile([128, NBLK * NWIN], F32)
    nc.sync.dma_start(glw_sb[:], io["i_glw"][:])
    gridx_sb = cp.tile([128, GRID // 16], I16)
    nc.sync.dma_start(gridx_sb[:], io["i_gridx"][:])
    ident_bf = cp.tile([128, 128], BF16)
    make_identity(nc, ident_bf[:])
    xo_sb = cp.tile([32, NSHP], BF16)
    nc.sync.dma_start(xo_sb[:], io["i_xTown"][:])
    h1own = cp.tile([128, NBLK * 64], BF16)
    consts = _load_edge_consts(nc, tc, ctx, meta, dict(
        idx_src=io["i_isrc"], idx_dst=io["i_idst"], dstl=io["i_dstl"],
        iota_bf=io["i_iota"]), tg)

    tab1, adtab1 = io["tab1"], io["adtab1"]
    tab2, adtab2 = io["tab2"], io["adtab2"]
    h1locT, h1Tfull = io["h1locT"], io["h1Tfull"]
    winbuf, grid, gmine = io["winbuf"], io["grid"], io["gmine"]

    # ---------------- layer-1 adtab (own shard) + replicated full table
    with ExitStack() as tctx:
        psA = tctx.enter_context(tc.tile_pool(name=f"t1pa{tg}", bufs=2, space="PSUM"))
        rowp = tctx.enter_context(tc.tile_pool(name=f"t1row{tg}", bufs=3))
        for b in range(NBLK):
            dps = psA.tile([128, 8], F32, tag="dps")
            nc.tensor.matmul(dps[:], lhsT=xo_sb[:, b * 128:(b + 1) * 128],
                             rhs=wad1_sb[:, 0:8], start=True, stop=True)
            adrow = rowp.tile([128, 128], BF16, tag="adrow")
            nc.vector.memset(adrow[:, 8:128], 0.0)
            nc.vector.tensor_copy(adrow[:, 0:8], dps[:])
            nc.sync.dma_start(adtab1[b * 128:(b + 1) * 128, :], adrow[:])

    with ExitStack() as tctx:
        ps5 = tctx.enter_context(tc.tile_pool(name=f"t1p5{tg}", bufs=2, space="PSUM"))
        ps8 = tctx.enter_context(tc.tile_pool(name=f"t1p8{tg}", bufs=2, space="PSUM"))
        rowp = tctx.enter_context(tc.tile_pool(name=f"t1row{tg}b", bufs=4))
        xsp = tctx.enter_context(tc.tile_pool(name=f"t1xs{tg}", bufs=2))
        for cs in range(NCORES):
            xs = xsp.tile([32, NSHP], BF16, tag="xs")
            nc.sync.dma_start(xs[:], io["i_xT"][:, cs * NSHP:(cs + 1) * NSHP])
            for b in range(NBLK):
                lhsT = xs[:, b * 128:(b + 1) * 128]
                hps = ps5.tile([128, 512], F32, tag="hps")
                nc.tensor.matmul(hps[:], lhsT=lhsT, rhs=w1_sb[:, 0:512],
                                 start=True, stop=True)
                aps = ps8.tile([128, 8], F32, tag="aps")
                nc.tensor.matmul(aps[:], lhsT=lhsT, rhs=w1_sb[:, 512:520],
                                 start=True, stop=True)
                row = rowp.tile([128, 640], BF16, tag="row")
                if b % 2 == 0:
                    nc.scalar.copy(row[:, 0:512], hps[:])
                else:
                    nc.vector.tensor_copy(row[:, 0:512], hps[:])
                nc.vector.tensor_copy(row[:, 512:520], aps[:])
                r0 = cs * NSHP + b * 128
                nc.sync.dma_start(tab1[r0:r0 + 128, 0:520], row[:, 0:520])

    # ---------------- layer-1 edge pass -> h1own + h1locT + adtab2
    with ExitStack() as ectx:
        ep = ectx.enter_context(tc.tile_pool(name=f"ep1{tg}", bufs=3))
        psT = ectx.enter_context(tc.tile_pool(name=f"e1pt{tg}", bufs=2, space="PSUM"))
        psA = ectx.enter_context(tc.tile_pool(name=f"e1pa{tg}", bufs=2, space="PSUM"))
        htp = ectx.enter_context(tc.tile_pool(name=f"e1ht{tg}", bufs=2))

        def epil1(b, num_ps, den_ps):
            den = ep.tile([128, 8], F32, tag="den")
            nc.vector.tensor_scalar(out=den[:], in0=den_ps[:], scalar1=8.0,
                                    scalar2=1e-20, op0=mybir.AluOpType.mult,
                                    op1=mybir.AluOpType.add)
            rec = ep.tile([128, 8], F32, tag="rec")
            nc.vector.reciprocal(rec[:], den[:])
            tmp = ep.tile([128, 512], F32, tag="tmp")
            nc.vector.tensor_tensor(
                out=tmp[:].rearrange("p (h f) -> p h f", h=H),
                in0=num_ps[:].rearrange("p (h f) -> p h f", h=H),
                in1=rec[:].unsqueeze(2).to_broadcast([128, H, F]),
                op=mybir.AluOpType.mult)
            t3 = tmp[:].rearrange("p (h f) -> p h f", h=H)
            a4 = ep.tile([128, 256], F32, tag="a4")
            nc.vector.tensor_tensor(
                out=a4[:].rearrange("p (h f) -> p h f", h=4),
                in0=t3[:, 0:4, :], in1=t3[:, 4:8, :], op=mybir.AluOpType.add)
            a4v = a4[:].rearrange("p (h f) -> p h f", h=4)
            a2 = ep.tile([128, 128], F32, tag="a2")
            nc.vector.tensor_tensor(
                out=a2[:].rearrange("p (h f) -> p h f", h=2),
                in0=a4v[:, 0:2, :], in1=a4v[:, 2:4, :], op=mybir.AluOpType.add)
            a2v = a2[:].rearrange("p (h f) -> p h f", h=2)
            a1 = ep.tile([128, 64], F32, tag="a1")
            nc.vector.tensor_tensor(out=a1[:], in0=a2v[:, 0, :], in1=a2v[:, 1, :],
                                    op=mybir.AluOpType.add)
            nc.vector.tensor_tensor(out=h1own[:, b * 64:(b + 1) * 64],
                                    in0=a1[:], in1=b1_sb[:],
                                    op=mybir.AluOpType.add)
            # transpose h1 block -> h1locT column block + layer-2 adtab row
            tps = psT.tile([64, 128], BF16, tag="tps")
            nc.tensor.transpose(tps[:], h1own[:, b * 64:(b + 1) * 64], ident_bf[:])
            hT = htp.tile([64, 128], BF16, tag="hT")
            nc.scalar.copy(hT[:], tps[:])
            nc.sync.dma_start(h1locT[:, b * 128:(b + 1) * 128], hT[:])
            dps = psA.tile([128, 8], F32, tag="dps")
            nc.tensor.matmul(dps[:], lhsT=hT[:], rhs=wad2_sb[:, 0:8],
                             start=True, stop=True)
            adrow = ep.tile([128, 128], BF16, tag="adrow")
            nc.vector.memset(adrow[:, 8:128], 0.0)
            nc.vector.tensor_copy(adrow[:, 0:8], dps[:])
            nc.sync.dma_start(adtab2[b * 128:(b + 1) * 128, :], adrow[:])

        _edge_pass(nc, tc, ectx, meta, tab1[0:HALF, :],
                   tab1[HALF:NPADN, :], adtab1, f"1{tg}", consts, epil1)

    # ---------------- AllGather h1^T (bf16, 0.8MB per core)
    nc.gpsimd.collective_compute(
        "AllGather", mybir.AluOpType.bypass, replica_groups=groups,
        ins=[h1locT[:].opt()], outs=[h1Tfull[:].opt()])

    # ---------------- layer-2 replicated full table
    with ExitStack() as tctx:
        ps5 = tctx.enter_context(tc.tile_pool(name=f"t2p5{tg}", bufs=2, space="PSUM"))
        ps8 = tctx.enter_context(tc.tile_pool(name=f"t2p8{tg}", bufs=2, space="PSUM"))
        rowp = tctx.enter_context(tc.tile_pool(name=f"t2row{tg}", bufs=4))
        hsp = tctx.enter_context(tc.tile_pool(name=f"t2hs{tg}", bufs=2))
        for cs in range(NCORES):
            hs = hsp.tile([64, NSHP], BF16, tag="hs")
            nc.sync.dma_start(hs[:], h1Tfull[cs * 64:(cs + 1) * 64, :])
            for b in range(NBLK):
                lhsT = hs[:, b * 128:(b + 1) * 128]
                hps = ps5.tile([128, 512], F32, tag="hps")
                nc.tensor.matmul(hps[:], lhsT=lhsT, rhs=w2_sb[:, 0:512],
                                 start=True, stop=True)
                aps = ps8.tile([128, 8], F32, tag="aps")
                nc.tensor.matmul(aps[:], lhsT=lhsT, rhs=w2_sb[:, 512:520],
                                 start=True, stop=True)
                row = rowp.tile([128, 640], BF16, tag="row")
                if b % 2 == 0:
                    nc.scalar.copy(row[:, 0:512], hps[:])
                else:
                    nc.vector.tensor_copy(row[:, 0:512], hps[:])
                nc.vector.tensor_copy(row[:, 512:520], aps[:])
                r0 = cs * NSHP + b * 128
                nc.sync.dma_start(tab2[r0:r0 + 128, 0:520], row[:, 0:520])

    # ---------------- layer-2 edge pass + window pooling
    with ExitStack() as ectx:
        ep = ectx.enter_context(tc.tile_pool(name=f"ep2{tg}", bufs=3))
        sgp = ectx.enter_context(tc.tile_pool(name=f"sg{tg}", bufs=3))
        psG = ectx.enter_context(tc.tile_pool(name=f"psg{tg}", bufs=1, space="PSUM"))
        gw_ps = []
        for w in range(NWIN):
            gw_tile = psG.tile([128, 512], F32, tag=f"gw{w}")
            gw_ps.append(gw_tile)

        def epil2(b, num_ps, den_ps):
            den = ep.tile([128, 8], F32, tag="den")
            nc.vector.tensor_scalar(out=den[:], in0=den_ps[:], scalar1=1e-20,
                                    scalar2=None, op0=mybir.AluOpType.add)
            rec = ep.tile([128, 8], F32, tag="rec")
            nc.vector.reciprocal(rec[:], den[:])
            o2f = ep.tile([128, 512], F32, tag="o2f")
            nc.vector.tensor_tensor(
                out=o2f[:].rearrange("p (h f) -> p h f", h=H),
                in0=num_ps[:].rearrange("p (h f) -> p h f", h=H),
                in1=rec[:].unsqueeze(2).to_broadcast([128, H, F]),
                op=mybir.AluOpType.mult)
            o2 = ep.tile([128, 512], BF16, tag="o2")
            nc.vector.tensor_tensor(out=o2[:], in0=o2f[:], in1=b2_sb[:],
                                    op=mybir.AluOpType.add)
            for w in range(NWIN):
                Sg = sgp.tile([128, 128], BF16, tag="Sg")
                nc.vector.tensor_scalar(
                    out=Sg[:], in0=consts["iota_bf"][:],
                    scalar1=glw_sb[:, b * NWIN + w:b * NWIN + w + 1],
                    scalar2=None, op0=mybir.AluOpType.is_equal)
                nc.tensor.matmul(gw_ps[w][:], lhsT=Sg[:], rhs=o2[:],
                                 start=(b == 0), stop=(b == NBLK - 1))

        _edge_pass(nc, tc, ectx, meta, tab2[0:HALF, :],
                   tab2[HALF:NPADN, :], adtab2, f"2{tg}", consts, epil2)

        # windows -> winbuf rows [0, 384); zero rows [384, 512)
        zt = ep.tile([128, 512], F32, tag="zt")
        nc.gpsimd.memset(zt[:], 0.0)
        nc.sync.dma_start(winbuf[NWIN * 128:WROWS, :], zt[:])
        for w in range(NWIN):
            wsb = ep.tile([128, 512], F32, tag="wsb")
            nc.vector.tensor_copy(wsb[:], gw_ps[w][:])
            nc.sync.dma_start(winbuf[w * 128:(w + 1) * 128, :], wsb[:])

    # ---------------- grid assembly + ReduceScatter
    with ExitStack() as gctx:
        gp = gctx.enter_context(tc.tile_pool(name=f"gridp{tg}", bufs=3))
        for t in range(GRID // 128):
            gtile = gp.tile([128, 1, 512], F32, tag="gtile")
            nc.gpsimd.dma_gather(
                out_ap=gtile[:], in_ap=winbuf[:],
                idxs_ap=gridx_sb[:, t * 8:(t + 1) * 8],
                num_idxs=128, num_idxs_reg=128, elem_size=512)
            nc.sync.dma_start(grid[t * 128:(t + 1) * 128, :],
                              gtile[:, 0, :])

    nc.gpsimd.collective_compute(
        "ReduceScatter", mybir.AluOpType.add, replica_groups=groups,
        ins=[grid[:].opt()], outs=[gmine[:].opt()])

    # ---------------- MLP on own 256 graphs
    with ExitStack() as mctx:
        mw = mctx.enter_context(tc.tile_pool(name=f"mw{tg}", bufs=1))
        fw1, fw2 = [], []
        for k in range(4):
            fw1_t = mw.tile([128, 512], BF16, tag=f"fw1{k}")
            fw1.append(fw1_t)
            fw2_t = mw.tile([128, 512], BF16, tag=f"fw2{k}")
            fw2.append(fw2_t)
        for k in range(4):
            nc.sync.dma_start(fw1[k][:], io["i_fw1"][k * 128:(k + 1) * 128, :])
            nc.sync.dma_start(fw2[k][:], io["i_fw2"][k * 128:(k + 1) * 128, :])
        fw3 = mw.tile([128, 4], BF16)
        nc.sync.dma_start(fw3[:], io["i_fw3"][:])
        fb1 = mw.tile([128, 4], F32)
        nc.sync.dma_start(fb1[:], io["i_fb1"][:])
        fb2 = mw.tile([128, 4], F32)
        nc.sync.dma_start(fb2[:], io["i_fb2"][:])
        fb3 = mw.tile([1, 1], F32)
        nc.sync.dma_start(fb3[:], io["i_fb3"][:])
        ident_f = mw.tile([128, 128], F32)
        make_identity(nc, ident_f[:])

        gp = mctx.enter_context(tc.tile_pool(name=f"mg{tg}", bufs=2))
        psT = mctx.enter_context(tc.tile_pool(name=f"mpt{tg}", bufs=2, space="PSUM"))
        psA = mctx.enter_context(tc.tile_pool(name=f"mpa{tg}", bufs=2, space="PSUM"))
        psO = mctx.enter_context(tc.tile_pool(name=f"mpo{tg}", bufs=2, space="PSUM"))
        ap_ = mctx.enter_context(tc.tile_pool(name=f"ma{tg}", bufs=2))

        for gt in range(GSH // 128):
            gl = gp.tile([128, 512], F32, tag="gl")
            nc.sync.dma_start(gl[:], gmine[gt * 128:(gt + 1) * 128, :])
            gTs = []
            for k in range(4):
                tps = psT.tile([128, 128], F32, tag="tps")
                nc.tensor.transpose(tps[:], gl[:, k * 128:(k + 1) * 128],
                                    ident_f[:])
                gT = ap_.tile([128, 128], BF16, tag=f"gT{k}")
                nc.vector.tensor_copy(gT[:], tps[:])
                gTs.append(gT)
            a1s, a2s = [], []
            for m in range(4):
                aps = psA.tile([128, 128], F32, tag="aps")
                for k in range(4):
                    nc.tensor.matmul(aps[:], lhsT=fw1[k][:, m * 128:(m + 1) * 128],
                                     rhs=gTs[k][:], start=(k == 0), stop=(k == 3))
                a1 = ap_.tile([128, 128], BF16, tag=f"a1{m}")
                nc.scalar.activation(a1[:], aps[:],
                                     mybir.ActivationFunctionType.Relu,
                                     bias=fb1[:, m:m + 1])
                a1s.append(a1)
            for m in range(4):
                aps = psA.tile([128, 128], F32, tag="bps")
                for k in range(4):
                    nc.tensor.matmul(aps[:], lhsT=fw2[k][:, m * 128:(m + 1) * 128],
                                     rhs=a1s[k][:], start=(k == 0), stop=(k == 3))
                a2 = ap_.tile([128, 128], BF16, tag=f"a2{m}")
                nc.scalar.activation(a2[:], aps[:],
                                     mybir.ActivationFunctionType.Relu,
                                     bias=fb2[:, m:m + 1])
                a2s.append(a2)
            ops = psO.tile([128, 128], F32, tag="ops")
            for k in range(4):
                nc.tensor.matmul(ops[0:1, :], lhsT=fw3[:, k:k + 1], rhs=a2s[k][:],
                                 start=(k == 0), stop=(k == 3))
            osb = ap_.tile([128, 128], F32, tag="osb")
            nc.scalar.activation(osb[0:1, :], ops[0:1, :],
                                 mybir.ActivationFunctionType.Identity,
                                 bias=fb3[0:1, 0:1])
            nc.sync.dma_start(io["o_out"][0:1, gt * 128:(gt + 1) * 128], osb[0:1, :])


def _build_merged(meta, iters=1):
    nc = bacc.Bacc("TRN2", target_bir_lowering=False, debug=False, num_devices=NCORES)
    IC = sum(nch * 8 for (_, _, nch) in meta["gathers"])
    groups = [list(range(NCORES))]

    io = {}
    io["i_xT"] = nc.dram_tensor("xT", [32, NPADN], BF16, kind="ExternalInput")
    io["i_xTown"] = nc.dram_tensor("xTown", [32, NSHP], BF16, kind="ExternalInput")
    io["i_w1"] = nc.dram_tensor("w1cat", [32, 520], BF16, kind="ExternalInput")
    io["i_wad1"] = nc.dram_tensor("wad1", [32, 8], BF16, kind="ExternalInput")
    io["i_b1"] = nc.dram_tensor("b1rep", [128, 64], F32, kind="ExternalInput")
    io["i_w2"] = nc.dram_tensor("w2cat", [64, 520], BF16, kind="ExternalInput")
    io["i_wad2"] = nc.dram_tensor("wad2", [64, 8], BF16, kind="ExternalInput")
    io["i_b2"] = nc.dram_tensor("b2rep", [128, 512], F32, kind="ExternalInput")
    io["i_isrc"] = nc.dram_tensor("idx_src", [128, IC], I16, kind="ExternalInput")
    io["i_idst"] = nc.dram_tensor("idx_dst", [128, IC], I16, kind="ExternalInput")
    io["i_dstl"] = nc.dram_tensor("dstl", [128, meta["TOTCH"]], F32, kind="ExternalInput")
    io["i_iota"] = nc.dram_tensor("iota_bf", [128, 128], BF16, kind="ExternalInput")
    io["i_glw"] = nc.dram_tensor("glw", [128, NBLK * NWIN], F32, kind="ExternalInput")
    io["i_gridx"] = nc.dram_tensor("gridx", [128, GRID // 16], I16, kind="ExternalInput")
    io["i_fw1"] = nc.dram_tensor("fcw1", [512, 512], BF16, kind="ExternalInput")
    io["i_fw2"] = nc.dram_tensor("fcw2", [512, 512], BF16, kind="ExternalInput")
    io["i_fw3"] = nc.dram_tensor("fcw3", [128, 4], BF16, kind="ExternalInput")
    io["i_fb1"] = nc.dram_tensor("fcb1", [128, 4], F32, kind="ExternalInput")
    io["i_fb2"] = nc.dram_tensor("fcb2", [128, 4], F32, kind="ExternalInput")
    io["i_fb3"] = nc.dram_tensor("fcb3", [1, 1], F32, kind="ExternalInput")
    io["o_out"] = nc.dram_tensor("out", [1, GSH], F32, kind="ExternalOutput")

    with tile.TileContext(nc, num_cores=NCORES) as tc:
        with ExitStack() as ctx:
            dram = ctx.enter_context(tc.tile_pool(name="dram", bufs=1, space="DRAM"))
            io["tab1"] = dram.tile([NPADN, 640], BF16, name="tab1")
            io["adtab1"] = dram.tile([NSHP, 128], BF16, name="adtab1")
            io["tab2"] = dram.tile([NPADN, 640], BF16, name="tab2")
            io["adtab2"] = dram.tile([NSHP, 128], BF16, name="adtab2")
            io["h1locT"] = dram.tile([64, NSHP], BF16, name="h1locT")
            io["h1Tfull"] = dram.tile([NCORES * 64, NSHP], BF16, addr_space="Shared", name="h1Tfull")
            io["winbuf"] = dram.tile([WROWS, 512], F32, name="winbuf")
            io["grid"] = dram.tile([GRID, 512], F32, name="grid")
            io["gmine"] = dram.tile([GSH, 512], F32, name="gmine")

            for it in range(iters):
                with ExitStack() as bctx:
                    _emit_iter(nc, tc, bctx, meta, io, groups, it)

    nc.compile()
    return nc


# ------------------------------------------------- cached PJRT runner
def _ensure_runner(nc, key):
    """Build the jitted shard_map executor for nc (once per program)."""
    import jax
    from jax.sharding import Mesh, PartitionSpec, NamedSharding
    from jax.experimental.shard_map import shard_map
    from concourse import bass2jax
    from concourse.bass2jax import _bass_exec_p, partition_id_tensor

    st = _cache.setdefault(key, {})
    if "fn" in st:
        return st
    bass2jax.install_neuronx_cc_hook()
    partition_name = (nc.partition_id_tensor.name
                      if nc.partition_id_tensor else None)
    in_names, out_names, out_avals = [], [], []
    for alloc in nc.m.functions[0].allocations:
        if not isinstance(alloc, mybir.MemoryLocationSet):
            continue
        name = alloc.memorylocations[0].name
        if alloc.kind == "ExternalInput":
            if name != partition_name:
                in_names.append(name)
        elif alloc.kind == "ExternalOutput":
            shape = tuple(alloc.tensor_shape)
            dtype = mybir.dt.np(alloc.dtype)
            out_names.append(name)
            out_avals.append(jax.core.ShapedArray(shape, dtype))
    n_params = len(in_names)
    all_names = list(in_names) + list(out_names)
    if partition_name is not None:
        all_names.append(partition_name)
    donate = tuple(range(n_params, n_params + len(out_names)))

    def _body(*args):
        operands = list(args)
        if partition_name is not None:
            operands.append(partition_id_tensor())
        outs = _bass_exec_p.bind(
            *operands, out_avals=tuple(out_avals), in_names=tuple(all_names),
            out_names=tuple(out_names), lowering_input_output_aliases=(),
            sim_require_finite=True, sim_require_nnan=True, nc=nc)
        return tuple(outs)

    devices = jax.devices()[:NCORES]
    mesh = Mesh(np.asarray(devices), ("core",))
    spec_in = (PartitionSpec("core"),) * (n_params + len(out_names))
    spec_out = (PartitionSpec("core"),) * len(out_names)
    fn = jax.jit(shard_map(_body, mesh=mesh, in_specs=spec_in,
                           out_specs=spec_out, check_rep=False),
                 donate_argnums=donate, keep_unused=True)
    st.update(fn=fn, in_names=in_names, out_names=out_names,
              out_avals=out_avals,
              shard=NamedSharding(mesh, PartitionSpec("core")))
    return st


def _host_prep(raws, meta):
    (x, edge_index, batch, W1, a_src1, a_dst1, b1, W2, a_src2, a_dst2,
     b2, fcW1, fcb1, fcW2, fcb2, fcW3, fcb3) = raws
    x = np.asarray(x, np.float32)
    xpad = np.zeros((NPADN, NF), np.float32)
    for c in range(NCORES):
        xpad[c * NSHP:c * NSHP + NSH] = x[c * NSH:(c + 1) * NSH]
    xT = np.ascontiguousarray(xpad.T).astype(BF)
    W1f = np.asarray(W1, np.float32)
    w1cat = np.concatenate(
        [W1f, _wcat(W1f, np.asarray(a_src1, np.float32))],
        axis=1).astype(BF)
    wad1 = _wcat(W1f, np.asarray(a_dst1, np.float32)).astype(BF)
    W2f = np.asarray(W2, np.float32)
    w2cat = np.concatenate(
        [W2f, _wcat(W2f, np.asarray(a_src2, np.float32))],
        axis=1).astype(BF)
    wad2 = _wcat(W2f, np.asarray(a_dst2, np.float32)).astype(BF)
    b1rep = np.tile(np.asarray(b1, np.float32)[None, :], (128, 1))
    b2rep = np.tile(np.asarray(b2, np.float32)[None, :], (128, 1))
    iota_bf = np.tile(np.arange(128, dtype=np.float32), (128, 1)).astype(BF)
    # fcW1 rows permuted to f-major g layout
    fcW1p = np.asarray(fcW1, np.float32)
    fcb1a = np.asarray(fcb1, np.float32).reshape(4, 128).T.copy()
    fcb2a = np.asarray(fcb2, np.float32).reshape(4, 128).T.copy()
    fw3a = np.asarray(fcW3, np.float32).reshape(4, 128).T.astype(BF).copy()

    in_maps = []
    for c in range(NCORES):
        xTown = np.ascontiguousarray(
            xpad[c * NSHP:(c + 1) * NSHP].T).astype(BF)
        in_maps.append(dict(
            xT=xT, xTown=xTown, w1cat=w1cat, wad1=wad1, b1rep=b1rep,
            w2cat=w2cat, wad2=wad2, b2rep=b2rep,
            idx_src=meta["idx_src"][c], idx_dst=meta["idx_dst"][c],
            dstl=meta["dstl"][c], iota_bf=iota_bf, glw=meta["glw"][c],
            gridx=meta["gridx"][c],
            fcw1=fcW1p.astype(BF),
            fcw2=np.asarray(fcW2, np.float32).astype(BF), fcw3=fw3a,
            fcb1=fcb1a, fcb2=fcb2a,
            fcb3=np.asarray(fcb3, np.float32).reshape(1, 1)))
    return in_maps


# ----------------------------------------------------------------- kernel()
def kernel(x, edge_index, batch, W1, a_src1, a_dst1, b1, W2, a_src2, a_dst2, b2,
           fcW1, fcb1, fcW2, fcb2, fcW3, fcb3):
    import jax

    raws = (x, edge_index, batch, W1, a_src1, a_dst1, b1, W2, a_src2, a_dst2,
            b2, fcW1, fcb1, fcW2, fcb2, fcW3, fcb3)
    raws = tuple(np.asarray(a) for a in raws)
    cached = _cache.get("raws")
    same = [cached is not None and len(cached) == len(raws)
            and a.shape == b.shape and np.array_equal(a, b)
            for a, b in zip(raws, cached or raws)]
    hit = bool(same) and all(same)

    if not hit:
        graph_same = bool(same) and same[1] and same[2] and "meta" in _cache
        _cache["raws"] = tuple(np.array(a, copy=True) for a in raws)
        meta = _cache["meta"] if graph_same else _preprocess(
            np.asarray(raws[1]), np.asarray(raws[2]))
        _cache["meta"] = meta
        key = (meta["TOTCH"], len(meta["gathers"]), tuple(meta["g0"]))
        if _cache.get("progkey") != key:
            _cache["prog"] = _build_merged(meta)
            _cache["progkey"] = key
            _cache.pop("runner", None)
            _cache.pop("benchrunner", None)
            _cache.pop("benchprog", None)

        in_maps = _host_prep(raws, meta)
        _cache["in_maps"] = in_maps
        st = _ensure_runner(_cache["prog"], "runner")
        st["dev_args"] = [
            jax.device_put(
                np.concatenate([np.asarray(m[name]) for m in in_maps], axis=0),
                st["shard"])
            for name in st["in_names"]]

    st = _cache["runner"]
    zeros = [jax.device_put(
        np.zeros((NCORES * av.shape[0], *av.shape[1:]), av.dtype), st["shard"])
        for av in st["out_avals"]]

    t0 = time.time()
    outs = st["fn"](*st["dev_args"], *zeros)
    res = [np.asarray(o) for o in outs]
    LAST_TIMES.clear()
    LAST_TIMES["p"] = time.time() - t0

    oi = st["out_names"].index("out")
    out = res[oi].reshape(NCORES, GSH).reshape(-1)  # [2048] in core order
    return out[:G].astype(np.float32).reshape(G, 1)


# ----------------------------------------------------------- bench (K iters)
BENCH_ITERS = 4


def bench_call():
    """Run the K-iteration program once; returns (wall_s, out[G,1]).

    Requires kernel() to have been called at least once (device inputs
    cached). The K-iteration NEFF executes the full kernel K times
    back-to-back on device, so wall = RTT + K * t_exec.
    """
    import jax
    if "benchprog" not in _cache:
        _cache["benchprog"] = _build_merged(_cache["meta"], iters=BENCH_ITERS)
    st = _ensure_runner(_cache["benchprog"], "benchrunner")
    if "dev_args" not in st:
        in_maps = _cache["in_maps"]
        st["dev_args"] = [
            jax.device_put(
                np.concatenate([np.asarray(m[name]) for m in in_maps], axis=0),
                st["shard"])
            for name in st["in_names"]]
    zeros = [jax.device_put(
        np.zeros((NCORES * av.shape[0], *av.shape[1:]), av.dtype), st["shard"])
        for av in st["out_avals"]]
    t0 = time.time()
    outs = st["fn"](*st["dev_args"], *zeros)
    res = [np.asarray(o) for o in outs]
    wall = time.time() - t0
    oi = st["out_names"].index("out")
    out = res[oi].reshape(NCORES, GSH).reshape(-1)
    return wall, out[:G].astype(np.float32).reshape(G, 1)


# revision 7
# speedup vs baseline: 1.0785x; 1.0785x over previous
"""GAT (2-layer, 8-head) + graph pooling + MLP on 8 TRN2 NeuronCores.

Single merged SPMD program (one dispatch per call) with on-device collectives:
  - layer-1 gather table is REPLICATED: every core computes all 50176 rows
    from the full x (cheap [32,520] matmuls) -> no AllGather-1
  - layer-2: AllGather only the bf16 h1^T [64, 6272] per core (0.8MB), then
    every core computes the full layer-2 table locally
  - tables use f-major head layout (row = [f0h0..f0h7, f1h0..]) so the
    per-edge msg multiply qualifies for DVE 2x_1P mode
  - per layer: edge pass with dma_gather by src (640-wide bf16 rows),
    attention ex = exp(leakyrelu(as+ad)), one-hot S matmuls for
    segment-softmax sum/den per 128-dst block
  - layer-2 epilogue pools nodes into 3 local 128-graph windows, windows
    land in a [2048, 512] global graph grid via dma_gather, then a
    ReduceScatter hands core c the pooled rows [256c, 256c+256)
  - each core runs the small MLP on its 256 graphs -> out [1, 256]

`_build_merged(meta, iters=K)` unrolls the whole kernel K times inside one
NEFF (same buffers) so test.py can subtract the host-dispatch RTT:
t_exec = (wall(K) - wall(1)) / (K - 1).
"""
import os
import time
import numpy as np
import ml_dtypes
from contextlib import ExitStack

os.environ.setdefault("JAX_PLATFORMS", "")  # allow axon platform auto-detect

LAST_TIMES = {}

import concourse.bass as bass
import concourse.bacc as bacc
import concourse.mybir as mybir
import concourse.tile as tile
from concourse.masks import make_identity

BF = ml_dtypes.bfloat16

N = 50000
E = 800000
NF = 32
H = 8
F = 64
HF = 512
G = 2000
NEG = 0.2
NCORES = 8
NSH = N // NCORES            # 6250 dst nodes per core
NBLK = (NSH + 127) // 128    # 49 blocks
NSHP = NBLK * 128            # 6272
NPADN = NCORES * NSHP        # 50176 padded-global rows (core-major)
HALF = 4 * NSHP              # 25088 = cores 0-3 -> table A (int16 idx range)
NWIN = 3                     # 128-graph windows per core (local range < 384)
WROWS = 512                  # winbuf rows: 384 window rows + zero pad
ZROW = 400                   # a guaranteed-zero winbuf row for out-of-range
GRID = 2048                  # global graph grid rows (>= G, 16 tiles of 128)
GSH = GRID // NCORES         # 256 graphs per core after ReduceScatter

F32 = mybir.dt.float32
BF16 = mybir.dt.bfloat16
I16 = mybir.dt.int16

_cache = {}


# ---------------------------------------------------------------- host prep
def _preprocess(edge_index, batch):
    src = np.concatenate([edge_index[0], np.arange(N, dtype=np.int64)]).astype(np.int64)
    dst = np.concatenate([edge_index[1], np.arange(N, dtype=np.int64)]).astype(np.int64)
    core = dst // NSH
    dloc = (dst - core * NSH).astype(np.int64)
    blk = dloc // 128
    # padded-global table row of the source node (core-major, stride NSHP)
    spad = (src // NSH) * NSHP + (src % NSH)
    tab = (spad >= HALF).astype(np.int64)

    # group key per edge: (core, blk, tab)
    key = (core * NBLK + blk) * 2 + tab
    order = np.argsort(key, kind="stable")
    src_s, dst_s, key_s = spad[order], dst[order], key[order]
    counts = np.bincount(key_s, minlength=NCORES * NBLK * 2).reshape(NCORES, NBLK * 2)

    # uniform chunk counts across cores
    K = np.ceil(counts.max(axis=0) / 128.0).astype(np.int64)  # [NBLK*2]
    TOTCH = int(K.sum())
    choff = np.concatenate([[0], np.cumsum(K)])  # chunk offset per group

    # per-core flat edge slot arrays [TOTCH*128]
    srci = np.zeros((NCORES, TOTCH * 128), np.int16)
    dsti = np.zeros((NCORES, TOTCH * 128), np.int16)
    dstl = np.full((NCORES, TOTCH * 128), -1.0, np.float32)

    gstart = np.concatenate([[0], np.cumsum(counts.reshape(-1))[:-1]])
    gs = gstart.reshape(NCORES, NBLK * 2)
    for c in range(NCORES):
        for g in range(NBLK * 2):
            n = counts[c, g]
            if n == 0:
                continue
            s0 = gs[c, g]
            es, ed = src_s[s0:s0 + n], dst_s[s0:s0 + n]
            o0 = choff[g] * 128
            t = g & 1
            srci[c, o0:o0 + n] = (es - t * HALF).astype(np.int16)
            dl = (ed - c * NSH).astype(np.int64)
            dsti[c, o0:o0 + n] = dl.astype(np.int16)
            dstl[c, o0:o0 + n] = (dl - (g // 2) * 128).astype(np.float32)

    # gather runs: per group, runs of <=8 chunks
    gathers = []  # (tab, chunk0, nch)
    for g in range(NBLK * 2):
        k = int(K[g])
        c0 = int(choff[g])
        while k > 0:
            nch = min(k, 8)
            gathers.append((g & 1, c0, nch))
            c0 += nch
            k -= nch

    def wrap16(v):  # [n] -> [128, n//16] column-major wrap, replicated
        n = v.shape[0]
        return np.tile(v.reshape(n // 16, 16).T, (8, 1)).astype(np.int16)

    idx_src = [np.concatenate(
        [wrap16(srci[c, c0 * 128:(c0 + nch) * 128]) for (_, c0, nch) in gathers], axis=1)
        for c in range(NCORES)]
    idx_dst = [np.concatenate(
        [wrap16(dsti[c, c0 * 128:(c0 + nch) * 128]) for (_, c0, nch) in gathers], axis=1)
        for c in range(NCORES)]
    dstl_t = [dstl[c].reshape(TOTCH, 128).T.copy() for c in range(NCORES)]

    # block boundaries in chunk space: block b covers chunks [choff[2b], choff[2b+2])
    blk_first = [int(choff[2 * b]) for b in range(NBLK)]
    blk_last = [int(choff[2 * b + 2]) - 1 for b in range(NBLK)]

    # graph-local window values per (lane, block, window)
    g0 = [int(batch[c * NSH]) for c in range(NCORES)]
    glw = []
    for c in range(NCORES):
        gmax = int(batch[(c + 1) * NSH - 1]) if c < NCORES - 1 else int(batch[N - 1])
        assert gmax - g0[c] < NWIN * 128, "graph-local id range exceeds windows"
        a = np.full((128, NBLK * NWIN), -1.0, np.float32)
        for b in range(NBLK):
            nn = min(128, NSH - b * 128)
            nodes = c * NSH + b * 128 + np.arange(nn)
            gl = batch[nodes] - g0[c]
            for w in range(NWIN):
                a[:nn, b * NWIN + w] = gl - 128 * w
        glw.append(a)

    # grid gather indices: grid row r <- winbuf row (r - g0) if in window
    # range else the zeroed row ZROW
    gridx = []
    for c in range(NCORES):
        r = np.arange(GRID, dtype=np.int64) - g0[c]
        idx = np.where((r >= 0) & (r < NWIN * 128), r, ZROW).astype(np.int16)
        gridx.append(np.concatenate(
            [np.tile(idx[t * 128:(t + 1) * 128].reshape(8, 16).T, (8, 1))
             for t in range(GRID // 128)], axis=1))

    return dict(TOTCH=TOTCH, gathers=gathers, blk_first=blk_first, blk_last=blk_last,
                idx_src=idx_src, idx_dst=idx_dst, dstl=dstl_t, glw=glw, g0=g0,
                gridx=gridx)


def _wcat(Wmat, a_vec):
    # [fin, H*F] weight + per-head attention vec -> [fin, H] alpha weight
    fin = Wmat.shape[0]
    Wr = Wmat.reshape(fin, H, F)
    return np.einsum("fhk,hk->fh", Wr, a_vec)


def _fmajor(Wmat):
    # [fin, H*F] h-major columns -> f-major columns [fin, F*H]
    fin = Wmat.shape[0]
    return Wmat.reshape(fin, H, F).transpose(0, 2, 1).reshape(fin, H * F)


# ------------------------------------------------------------- device build
def _edge_pass(nc, tc, ctx, meta, tabA, tabB, adtab, tagb, consts, epil):
    """Shared edge-processing pass. epil(b, num_ps, den_ps) emits the block
    epilogue after the block's last chunk. f-major head layout throughout."""
    IC_off = 0
    sb = ctx.enter_context(tc.tile_pool(name=f"eg{tagb}", bufs=3))
    sbm = ctx.enter_context(tc.tile_pool(name=f"em{tagb}", bufs=6))
    psN = ctx.enter_context(tc.tile_pool(name=f"pn{tagb}", bufs=2, space="PSUM"))
    psD = ctx.enter_context(tc.tile_pool(name=f"pd{tagb}", bufs=2, space="PSUM"))

    iota_bf = consts["iota_bf"]
    dstl_sb = consts["dstl_sb"]
    isrc_sb = consts["isrc_sb"]
    idst_sb = consts["idst_sb"]

    num_ps = den_ps = None
    cur_blk = -1
    for (t, c0, nch) in meta["gathers"]:
        n = nch * 128
        cols = nch * 8
        gt = sb.tile([128, nch, 640], BF16, tag="maing")
        nc.gpsimd.dma_gather(
            out_ap=gt[:], in_ap=(tabA if t == 0 else tabB),
            idxs_ap=isrc_sb[:, IC_off:IC_off + cols],
            num_idxs=n, num_idxs_reg=n, elem_size=640)
        adt = sb.tile([128, nch, 128], BF16, tag="adg")
        nc.gpsimd.dma_gather(
            out_ap=adt[:], in_ap=adtab[:],
            idxs_ap=idst_sb[:, IC_off:IC_off + cols],
            num_idxs=n, num_idxs_reg=n, elem_size=128)
        IC_off += cols

        e_st = sb.tile([128, nch, 8], F32, tag="est")
        nc.vector.tensor_tensor(
            out=e_st[:], in0=gt[:, :, 512:520], in1=adt[:, :, 0:8],
            op=mybir.AluOpType.add)
        e_fl = e_st[:].rearrange("p a b -> p (a b)")
        t_sc = sb.tile([128, nch * 8], F32, tag="esc")
        nc.vector.tensor_scalar(out=t_sc[:], in0=e_fl, scalar1=NEG, scalar2=None,
                                op0=mybir.AluOpType.mult)
        nc.vector.tensor_tensor(out=e_fl, in0=e_fl, in1=t_sc[:],
                                op=mybir.AluOpType.max)
        ex_st = sb.tile([128, nch * 8], BF16, tag="exs")
        nc.scalar.activation(ex_st[:], e_fl, mybir.ActivationFunctionType.Exp)

        for j in range(nch):
            ch = c0 + j
            if num_ps is None or ch > meta["blk_last"][cur_blk]:
                cur_blk += 1
                num_ps = psN.tile([128, 512], F32, tag="nps")
                den_ps = psD.tile([128, 8], F32, tag="dps")
            S = sbm.tile([128, 128], BF16, tag="S")
            nc.vector.tensor_scalar(
                out=S[:], in0=iota_bf[:], scalar1=dstl_sb[:, ch:ch + 1],
                scalar2=None, op0=mybir.AluOpType.is_equal)
            msg = sbm.tile([128, 512], BF16, tag="msg")
            nc.vector.tensor_tensor(
                out=msg[:].rearrange("p (h f) -> p h f", h=H),
                in0=gt[:, j, 0:512].rearrange("p (h f) -> p h f", h=H),
                in1=ex_st[:, 8 * j:8 * j + 8].unsqueeze(2).to_broadcast([128, H, F]),
                op=mybir.AluOpType.mult)
            first = ch == meta["blk_first"][cur_blk]
            last = ch == meta["blk_last"][cur_blk]
            nc.tensor.matmul(num_ps[:], lhsT=S[:], rhs=msg[:], start=first, stop=last)
            nc.tensor.matmul(den_ps[:], lhsT=S[:], rhs=ex_st[:, 8 * j:8 * j + 8],
                             start=first, stop=last)
            if last:
                epil(cur_blk, num_ps, den_ps)


def _load_edge_consts(nc, tc, ctx, meta, inp, tagb):
    consts = {}
    cp = ctx.enter_context(tc.tile_pool(name=f"ec{tagb}", bufs=1))
    IC = sum(nch * 8 for (_, _, nch) in meta["gathers"])
    isrc_sb = cp.tile([128, IC], I16)
    nc.sync.dma_start(isrc_sb[:], inp["idx_src"][:])
    idst_sb = cp.tile([128, IC], I16)
    nc.sync.dma_start(idst_sb[:], inp["idx_dst"][:])
    dstl_sb = cp.tile([128, meta["TOTCH"]], F32)
    nc.sync.dma_start(dstl_sb[:], inp["dstl"][:])
    iota_bf = cp.tile([128, 128], BF16)
    nc.sync.dma_start(iota_bf[:], inp["iota_bf"][:])
    consts.update(isrc_sb=isrc_sb, idst_sb=idst_sb, dstl_sb=dstl_sb, iota_bf=iota_bf)
    return consts


def _emit_iter(nc, tc, ctx, meta, io, groups, it):
    """Emit one full kernel iteration. io: dict of dram tensors."""
    tg = f"_{it}"
    cp = ctx.enter_context(tc.tile_pool(name=f"wc{tg}", bufs=1))
    w1_sb = cp.tile([32, 520], BF16)
    nc.sync.dma_start(w1_sb[:], io["i_w1"][:])
    wad1_sb = cp.tile([32, 8], BF16)
    nc.sync.dma_start(wad1_sb[:], io["i_wad1"][:])
    b1_sb = cp.tile([128, 64], F32)
    nc.sync.dma_start(b1_sb[:], io["i_b1"][:])
    w2_sb = cp.tile([64, 520], BF16)
    nc.sync.dma_start(w2_sb[:], io["i_w2"][:])
    wad2_sb = cp.tile([64, 8], BF16)
    nc.sync.dma_start(wad2_sb[:], io["i_wad2"][:])
    b2_sb = cp.tile([128, 512], F32)
    nc.sync.dma_start(b2_sb[:], io["i_b2"][:])
    glw_sb = cp.tile([128, NBLK * NWIN], F32)
    nc.sync.dma_start(glw_sb[:], io["i_glw"][:])
    gridx_sb = cp.tile([128, GRID // 16], I16)
    nc.sync.dma_start(gridx_sb[:], io["i_gridx"][:])
    ident_bf = cp.tile([128, 128], BF16)
    make_identity(nc, ident_bf[:])
    xo_sb = cp.tile([32, NSHP], BF16)
    nc.sync.dma_start(xo_sb[:], io["i_xTown"][:])
    h1own = cp.tile([128, NBLK * 64], BF16)
    consts = _load_edge_consts(nc, tc, ctx, meta, dict(
        idx_src=io["i_isrc"], idx_dst=io["i_idst"], dstl=io["i_dstl"],
        iota_bf=io["i_iota"]), tg)

    tab1, adtab1 = io["tab1"], io["adtab1"]
    tab2, adtab2 = io["tab2"], io["adtab2"]
    h1locT, h1Tfull = io["h1locT"], io["h1Tfull"]
    winbuf, grid, gmine = io["winbuf"], io["grid"], io["gmine"]

    # ---------------- layer-1 adtab (own shard) + replicated full table
    with ExitStack() as tctx:
        psA = tctx.enter_context(tc.tile_pool(name=f"t1pa{tg}", bufs=2, space="PSUM"))
        rowp = tctx.enter_context(tc.tile_pool(name=f"t1row{tg}", bufs=3))
        for b in range(NBLK):
            dps = psA.tile([128, 8], F32, tag="dps")
            nc.tensor.matmul(dps[:], lhsT=xo_sb[:, b * 128:(b + 1) * 128],
                             rhs=wad1_sb[:, 0:8], start=True, stop=True)
            adrow = rowp.tile([128, 128], BF16, tag="adrow")
            nc.vector.memset(adrow[:, 8:128], 0.0)
            nc.vector.tensor_copy(adrow[:, 0:8], dps[:])
            nc.sync.dma_start(adtab1[b * 128:(b + 1) * 128, :], adrow[:])

    with ExitStack() as tctx:
        ps5 = tctx.enter_context(tc.tile_pool(name=f"t1p5{tg}", bufs=2, space="PSUM"))
        ps8 = tctx.enter_context(tc.tile_pool(name=f"t1p8{tg}", bufs=2, space="PSUM"))
        rowp = tctx.enter_context(tc.tile_pool(name=f"t1row{tg}b", bufs=4))
        xsp = tctx.enter_context(tc.tile_pool(name=f"t1xs{tg}", bufs=2))
        for cs in range(NCORES):
            xs = xsp.tile([32, NSHP], BF16, tag="xs")
            nc.sync.dma_start(xs[:], io["i_xT"][:, cs * NSHP:(cs + 1) * NSHP])
            for b in range(NBLK):
                lhsT = xs[:, b * 128:(b + 1) * 128]
                hps = ps5.tile([128, 512], F32, tag="hps")
                nc.tensor.matmul(hps[:], lhsT=lhsT, rhs=w1_sb[:, 0:512],
                                 start=True, stop=True)
                aps = ps8.tile([128, 8], F32, tag="aps")
                nc.tensor.matmul(aps[:], lhsT=lhsT, rhs=w1_sb[:, 512:520],
                                 start=True, stop=True)
                row = rowp.tile([128, 640], BF16, tag="row")
                if b % 2 == 0:
                    nc.scalar.copy(row[:, 0:512], hps[:])
                else:
                    nc.vector.tensor_copy(row[:, 0:512], hps[:])
                nc.vector.tensor_copy(row[:, 512:520], aps[:])
                r0 = cs * NSHP + b * 128
                nc.sync.dma_start(tab1[r0:r0 + 128, 0:520], row[:, 0:520])

    # ---------------- layer-1 edge pass -> h1own + h1locT + adtab2
    with ExitStack() as ectx:
        ep = ectx.enter_context(tc.tile_pool(name=f"ep1{tg}", bufs=3))
        psT = ectx.enter_context(tc.tile_pool(name=f"e1pt{tg}", bufs=2, space="PSUM"))
        psA = ectx.enter_context(tc.tile_pool(name=f"e1pa{tg}", bufs=2, space="PSUM"))
        htp = ectx.enter_context(tc.tile_pool(name=f"e1ht{tg}", bufs=2))

        def epil1(b, num_ps, den_ps):
            den = ep.tile([128, 8], F32, tag="den")
            nc.vector.tensor_scalar(out=den[:], in0=den_ps[:], scalar1=8.0,
                                    scalar2=1e-20, op0=mybir.AluOpType.mult,
                                    op1=mybir.AluOpType.add)
            rec = ep.tile([128, 8], F32, tag="rec")
            nc.vector.reciprocal(rec[:], den[:])
            tmp = ep.tile([128, 512], F32, tag="tmp")
            nc.vector.tensor_tensor(
                out=tmp[:].rearrange("p (h f) -> p h f", h=H),
                in0=num_ps[:].rearrange("p (h f) -> p h f", h=H),
                in1=rec[:].unsqueeze(2).to_broadcast([128, H, F]),
                op=mybir.AluOpType.mult)
            t3 = tmp[:].rearrange("p (h f) -> p h f", h=H)
            a4 = ep.tile([128, 256], F32, tag="a4")
            nc.vector.tensor_tensor(
                out=a4[:].rearrange("p (h f) -> p h f", h=4),
                in0=t3[:, 0:4, :], in1=t3[:, 4:8, :], op=mybir.AluOpType.add)
            a4v = a4[:].rearrange("p (h f) -> p h f", h=4)
            a2 = ep.tile([128, 128], F32, tag="a2")
            nc.vector.tensor_tensor(
                out=a2[:].rearrange("p (h f) -> p h f", h=2),
                in0=a4v[:, 0:2, :], in1=a4v[:, 2:4, :], op=mybir.AluOpType.add)
            a2v = a2[:].rearrange("p (h f) -> p h f", h=2)
            a1 = ep.tile([128, 64], F32, tag="a1")
            nc.vector.tensor_tensor(out=a1[:], in0=a2v[:, 0, :], in1=a2v[:, 1, :],
                                    op=mybir.AluOpType.add)
            nc.vector.tensor_tensor(out=h1own[:, b * 64:(b + 1) * 64],
                                    in0=a1[:], in1=b1_sb[:],
                                    op=mybir.AluOpType.add)
            # transpose h1 block -> h1locT column block + layer-2 adtab row
            tps = psT.tile([64, 128], BF16, tag="tps")
            nc.tensor.transpose(tps[:], h1own[:, b * 64:(b + 1) * 64], ident_bf[:])
            hT = htp.tile([64, 128], BF16, tag="hT")
            nc.scalar.copy(hT[:], tps[:])
            nc.sync.dma_start(h1locT[:, b * 128:(b + 1) * 128], hT[:])
            dps = psA.tile([128, 8], F32, tag="dps")
            nc.tensor.matmul(dps[:], lhsT=hT[:], rhs=wad2_sb[:, 0:8],
                             start=True, stop=True)
            adrow = ep.tile([128, 128], BF16, tag="adrow")
            nc.vector.memset(adrow[:, 8:128], 0.0)
            nc.vector.tensor_copy(adrow[:, 0:8], dps[:])
            nc.sync.dma_start(adtab2[b * 128:(b + 1) * 128, :], adrow[:])

        _edge_pass(nc, tc, ectx, meta, tab1[0:HALF, :],
                   tab1[HALF:NPADN, :], adtab1, f"1{tg}", consts, epil1)

    # ---------------- AllGather h1^T (bf16, 0.8MB per core)
    nc.gpsimd.collective_compute(
        "AllGather", mybir.AluOpType.bypass, replica_groups=groups,
        ins=[h1locT[:].opt()], outs=[h1Tfull[:].opt()])

    # ---------------- layer-2 replicated full table
    with ExitStack() as tctx:
        ps5 = tctx.enter_context(tc.tile_pool(name=f"t2p5{tg}", bufs=2, space="PSUM"))
        ps8 = tctx.enter_context(tc.tile_pool(name=f"t2p8{tg}", bufs=2, space="PSUM"))
        rowp = tctx.enter_context(tc.tile_pool(name=f"t2row{tg}", bufs=4))
        hsp = tctx.enter_context(tc.tile_pool(name=f"t2hs{tg}", bufs=2))
        for cs in range(NCORES):
            hs = hsp.tile([64, NSHP], BF16, tag="hs")
            nc.sync.dma_start(hs[:], h1Tfull[cs * 64:(cs + 1) * 64, :])
            for b in range(NBLK):
                lhsT = hs[:, b * 128:(b + 1) * 128]
                hps = ps5.tile([128, 512], F32, tag="hps")
                nc.tensor.matmul(hps[:], lhsT=lhsT, rhs=w2_sb[:, 0:512],
                                 start=True, stop=True)
                aps = ps8.tile([128, 8], F32, tag="aps")
                nc.tensor.matmul(aps[:], lhsT=lhsT, rhs=w2_sb[:, 512:520],
                                 start=True, stop=True)
                row = rowp.tile([128, 640], BF16, tag="row")
                if b % 2 == 0:
                    nc.scalar.copy(row[:, 0:512], hps[:])
                else:
                    nc.vector.tensor_copy(row[:, 0:512], hps[:])
                nc.vector.tensor_copy(row[:, 512:520], aps[:])
                r0 = cs * NSHP + b * 128
                nc.sync.dma_start(tab2[r0:r0 + 128, 0:520], row[:, 0:520])

    # ---------------- layer-2 edge pass + window pooling
    with ExitStack() as ectx:
        ep = ectx.enter_context(tc.tile_pool(name=f"ep2{tg}", bufs=3))
        sgp = ectx.enter_context(tc.tile_pool(name=f"sg{tg}", bufs=3))
        psG = ectx.enter_context(tc.tile_pool(name=f"psg{tg}", bufs=1, space="PSUM"))
        gw_ps = []
        for w in range(NWIN):
            gw_tile = psG.tile([128, 512], F32, tag=f"gw{w}")
            gw_ps.append(gw_tile)

        def epil2(b, num_ps, den_ps):
            den = ep.tile([128, 8], F32, tag="den")
            nc.vector.tensor_scalar(out=den[:], in0=den_ps[:], scalar1=1e-20,
                                    scalar2=None, op0=mybir.AluOpType.add)
            rec = ep.tile([128, 8], F32, tag="rec")
            nc.vector.reciprocal(rec[:], den[:])
            o2f = ep.tile([128, 512], F32, tag="o2f")
            nc.vector.tensor_tensor(
                out=o2f[:].rearrange("p (h f) -> p h f", h=H),
                in0=num_ps[:].rearrange("p (h f) -> p h f", h=H),
                in1=rec[:].unsqueeze(2).to_broadcast([128, H, F]),
                op=mybir.AluOpType.mult)
            o2 = ep.tile([128, 512], BF16, tag="o2")
            nc.vector.tensor_tensor(out=o2[:], in0=o2f[:], in1=b2_sb[:],
                                    op=mybir.AluOpType.add)
            for w in range(NWIN):
                Sg = sgp.tile([128, 128], BF16, tag="Sg")
                nc.vector.tensor_scalar(
                    out=Sg[:], in0=consts["iota_bf"][:],
                    scalar1=glw_sb[:, b * NWIN + w:b * NWIN + w + 1],
                    scalar2=None, op0=mybir.AluOpType.is_equal)
                nc.tensor.matmul(gw_ps[w][:], lhsT=Sg[:], rhs=o2[:],
                                 start=(b == 0), stop=(b == NBLK - 1))

        _edge_pass(nc, tc, ectx, meta, tab2[0:HALF, :],
                   tab2[HALF:NPADN, :], adtab2, f"2{tg}", consts, epil2)

        # windows -> winbuf rows [0, 384); zero rows [384, 512)
        zt = ep.tile([128, 512], F32, tag="zt")
        nc.gpsimd.memset(zt[:], 0.0)
        nc.sync.dma_start(winbuf[NWIN * 128:WROWS, :], zt[:])
        for w in range(NWIN):
            wsb = ep.tile([128, 512], F32, tag="wsb")
            nc.vector.tensor_copy(wsb[:], gw_ps[w][:])
            nc.sync.dma_start(winbuf[w * 128:(w + 1) * 128, :], wsb[:])

    # ---------------- grid assembly + ReduceScatter
    with ExitStack() as gctx:
        gp = gctx.enter_context(tc.tile_pool(name=f"gridp{tg}", bufs=3))
        for t in range(GRID // 128):
            gtile = gp.tile([128, 1, 512], F32, tag="gtile")
            nc.gpsimd.dma_gather(
                out_ap=gtile[:], in_ap=winbuf[:],
                idxs_ap=gridx_sb[:, t * 8:(t + 1) * 8],
                num_idxs=128, num_idxs_reg=128, elem_size=512)
            nc.sync.dma_start(grid[t * 128:(t + 1) * 128, :],
                              gtile[:, 0, :])

    nc.gpsimd.collective_compute(
        "ReduceScatter", mybir.AluOpType.add, replica_groups=groups,
        ins=[grid[:].opt()], outs=[gmine[:].opt()])

    # ---------------- MLP on own 256 graphs
    with ExitStack() as mctx:
        mw = mctx.enter_context(tc.tile_pool(name=f"mw{tg}", bufs=1))
        fw1, fw2 = [], []
        for k in range(4):
            fw1_t = mw.tile([128, 512], BF16, tag=f"fw1{k}")
            fw1.append(fw1_t)
            fw2_t = mw.tile([128, 512], BF16, tag=f"fw2{k}")
            fw2.append(fw2_t)
        for k in range(4):
            nc.sync.dma_start(fw1[k][:], io["i_fw1"][k * 128:(k + 1) * 128, :])
            nc.sync.dma_start(fw2[k][:], io["i_fw2"][k * 128:(k + 1) * 128, :])
        fw3 = mw.tile([128, 4], BF16)
        nc.sync.dma_start(fw3[:], io["i_fw3"][:])
        fb1 = mw.tile([128, 4], F32)
        nc.sync.dma_start(fb1[:], io["i_fb1"][:])
        fb2 = mw.tile([128, 4], F32)
        nc.sync.dma_start(fb2[:], io["i_fb2"][:])
        fb3 = mw.tile([1, 1], F32)
        nc.sync.dma_start(fb3[:], io["i_fb3"][:])
        ident_f = mw.tile([128, 128], F32)
        make_identity(nc, ident_f[:])

        gp = mctx.enter_context(tc.tile_pool(name=f"mg{tg}", bufs=2))
        psT = mctx.enter_context(tc.tile_pool(name=f"mpt{tg}", bufs=2, space="PSUM"))
        psA = mctx.enter_context(tc.tile_pool(name=f"mpa{tg}", bufs=2, space="PSUM"))
        psO = mctx.enter_context(tc.tile_pool(name=f"mpo{tg}", bufs=2, space="PSUM"))
        ap_ = mctx.enter_context(tc.tile_pool(name=f"ma{tg}", bufs=2))

        for gt in range(GSH // 128):
            gl = gp.tile([128, 512], F32, tag="gl")
            nc.sync.dma_start(gl[:], gmine[gt * 128:(gt + 1) * 128, :])
            gTs = []
            for k in range(4):
                tps = psT.tile([128, 128], F32, tag="tps")
                nc.tensor.transpose(tps[:], gl[:, k * 128:(k + 1) * 128],
                                    ident_f[:])
                gT = ap_.tile([128, 128], BF16, tag=f"gT{k}")
                nc.vector.tensor_copy(gT[:], tps[:])
                gTs.append(gT)
            a1s, a2s = [], []
            for m in range(4):
                aps = psA.tile([128, 128], F32, tag="aps")
                for k in range(4):
                    nc.tensor.matmul(aps[:], lhsT=fw1[k][:, m * 128:(m + 1) * 128],
                                     rhs=gTs[k][:], start=(k == 0), stop=(k == 3))
                a1 = ap_.tile([128, 128], BF16, tag=f"a1{m}")
                nc.scalar.activation(a1[:], aps[:],
                                     mybir.ActivationFunctionType.Relu,
                                     bias=fb1[:, m:m + 1])
                a1s.append(a1)
            for m in range(4):
                aps = psA.tile([128, 128], F32, tag="bps")
                for k in range(4):
                    nc.tensor.matmul(aps[:], lhsT=fw2[k][:, m * 128:(m + 1) * 128],
                                     rhs=a1s[k][:], start=(k == 0), stop=(k == 3))
                a2 = ap_.tile([128, 128], BF16, tag=f"a2{m}")
                nc.scalar.activation(a2[:], aps[:],
                                     mybir.ActivationFunctionType.Relu,
                                     bias=fb2[:, m:m + 1])
                a2s.append(a2)
            ops = psO.tile([128, 128], F32, tag="ops")
            for k in range(4):
                nc.tensor.matmul(ops[0:1, :], lhsT=fw3[:, k:k + 1], rhs=a2s[k][:],
                                 start=(k == 0), stop=(k == 3))
            osb = ap_.tile([128, 128], F32, tag="osb")
            nc.scalar.activation(osb[0:1, :], ops[0:1, :],
                                 mybir.ActivationFunctionType.Identity,
                                 bias=fb3[0:1, 0:1])
            nc.sync.dma_start(io["o_out"][0:1, gt * 128:(gt + 1) * 128], osb[0:1, :])


def _build_merged(meta, iters=1):
    nc = bacc.Bacc("TRN2", target_bir_lowering=False, debug=False, num_devices=NCORES)
    IC = sum(nch * 8 for (_, _, nch) in meta["gathers"])
    groups = [list(range(NCORES))]

    io = {}
    io["i_xT"] = nc.dram_tensor("xT", [32, NPADN], BF16, kind="ExternalInput")
    io["i_xTown"] = nc.dram_tensor("xTown", [32, NSHP], BF16, kind="ExternalInput")
    io["i_w1"] = nc.dram_tensor("w1cat", [32, 520], BF16, kind="ExternalInput")
    io["i_wad1"] = nc.dram_tensor("wad1", [32, 8], BF16, kind="ExternalInput")
    io["i_b1"] = nc.dram_tensor("b1rep", [128, 64], F32, kind="ExternalInput")
    io["i_w2"] = nc.dram_tensor("w2cat", [64, 520], BF16, kind="ExternalInput")
    io["i_wad2"] = nc.dram_tensor("wad2", [64, 8], BF16, kind="ExternalInput")
    io["i_b2"] = nc.dram_tensor("b2rep", [128, 512], F32, kind="ExternalInput")
    io["i_isrc"] = nc.dram_tensor("idx_src", [128, IC], I16, kind="ExternalInput")
    io["i_idst"] = nc.dram_tensor("idx_dst", [128, IC], I16, kind="ExternalInput")
    io["i_dstl"] = nc.dram_tensor("dstl", [128, meta["TOTCH"]], F32, kind="ExternalInput")
    io["i_iota"] = nc.dram_tensor("iota_bf", [128, 128], BF16, kind="ExternalInput")
    io["i_glw"] = nc.dram_tensor("glw", [128, NBLK * NWIN], F32, kind="ExternalInput")
    io["i_gridx"] = nc.dram_tensor("gridx", [128, GRID // 16], I16, kind="ExternalInput")
    io["i_fw1"] = nc.dram_tensor("fcw1", [512, 512], BF16, kind="ExternalInput")
    io["i_fw2"] = nc.dram_tensor("fcw2", [512, 512], BF16, kind="ExternalInput")
    io["i_fw3"] = nc.dram_tensor("fcw3", [128, 4], BF16, kind="ExternalInput")
    io["i_fb1"] = nc.dram_tensor("fcb1", [128, 4], F32, kind="ExternalInput")
    io["i_fb2"] = nc.dram_tensor("fcb2", [128, 4], F32, kind="ExternalInput")
    io["i_fb3"] = nc.dram_tensor("fcb3", [1, 1], F32, kind="ExternalInput")
    io["o_out"] = nc.dram_tensor("out", [1, GSH], F32, kind="ExternalOutput")

    with tile.TileContext(nc, num_cores=NCORES) as tc:
        with ExitStack() as ctx:
            dram = ctx.enter_context(tc.tile_pool(name="dram", bufs=1, space="DRAM"))
            io["tab1"] = dram.tile([NPADN, 640], BF16, name="tab1")
            io["adtab1"] = dram.tile([NSHP, 128], BF16, name="adtab1")
            io["tab2"] = dram.tile([NPADN, 640], BF16, name="tab2")
            io["adtab2"] = dram.tile([NSHP, 128], BF16, name="adtab2")
            io["h1locT"] = dram.tile([64, NSHP], BF16, name="h1locT")
            io["winbuf"] = dram.tile([WROWS, 512], F32, name="winbuf")
            io["grid"] = dram.tile([GRID, 512], F32, name="grid")

            for it in range(iters):
                # collective outputs: single-writer rule -> one tile per iter
                io2 = dict(io)
                io2["h1Tfull"] = dram.tile(
                    [NCORES * 64, NSHP], BF16, addr_space="Shared",
                    name=f"h1Tfull{it}")
                io2["gmine"] = dram.tile([GSH, 512], F32, name=f"gmine{it}")
                with ExitStack() as bctx:
                    _emit_iter(nc, tc, bctx, meta, io2, groups, it)

    nc.compile()
    return nc


# ------------------------------------------------- cached PJRT runner
def _ensure_runner(nc, key):
    """Build the jitted shard_map executor for nc (once per program)."""
    import jax
    from jax.sharding import Mesh, PartitionSpec, NamedSharding
    from jax.experimental.shard_map import shard_map
    from concourse import bass2jax
    from concourse.bass2jax import _bass_exec_p, partition_id_tensor

    st = _cache.setdefault(key, {})
    if "fn" in st:
        return st
    bass2jax.install_neuronx_cc_hook()
    partition_name = (nc.partition_id_tensor.name
                      if nc.partition_id_tensor else None)
    in_names, out_names, out_avals = [], [], []
    for alloc in nc.m.functions[0].allocations:
        if not isinstance(alloc, mybir.MemoryLocationSet):
            continue
        name = alloc.memorylocations[0].name
        if alloc.kind == "ExternalInput":
            if name != partition_name:
                in_names.append(name)
        elif alloc.kind == "ExternalOutput":
            shape = tuple(alloc.tensor_shape)
            dtype = mybir.dt.np(alloc.dtype)
            out_names.append(name)
            out_avals.append(jax.core.ShapedArray(shape, dtype))
    n_params = len(in_names)
    all_names = list(in_names) + list(out_names)
    if partition_name is not None:
        all_names.append(partition_name)
    donate = tuple(range(n_params, n_params + len(out_names)))

    def _body(*args):
        operands = list(args)
        if partition_name is not None:
            operands.append(partition_id_tensor())
        outs = _bass_exec_p.bind(
            *operands, out_avals=tuple(out_avals), in_names=tuple(all_names),
            out_names=tuple(out_names), lowering_input_output_aliases=(),
            sim_require_finite=True, sim_require_nnan=True, nc=nc)
        return tuple(outs)

    devices = jax.devices()[:NCORES]
    mesh = Mesh(np.asarray(devices), ("core",))
    spec_in = (PartitionSpec("core"),) * (n_params + len(out_names))
    spec_out = (PartitionSpec("core"),) * len(out_names)
    fn = jax.jit(shard_map(_body, mesh=mesh, in_specs=spec_in,
                           out_specs=spec_out, check_rep=False),
                 donate_argnums=donate, keep_unused=True)
    st.update(fn=fn, in_names=in_names, out_names=out_names,
              out_avals=out_avals,
              shard=NamedSharding(mesh, PartitionSpec("core")))
    return st


def _host_prep(raws, meta):
    (x, edge_index, batch, W1, a_src1, a_dst1, b1, W2, a_src2, a_dst2,
     b2, fcW1, fcb1, fcW2, fcb2, fcW3, fcb3) = raws
    x = np.asarray(x, np.float32)
    xpad = np.zeros((NPADN, NF), np.float32)
    for c in range(NCORES):
        xpad[c * NSHP:c * NSHP + NSH] = x[c * NSH:(c + 1) * NSH]
    xT = np.ascontiguousarray(xpad.T).astype(BF)
    W1f = np.asarray(W1, np.float32)
    w1cat = np.concatenate(
        [W1f, _wcat(W1f, np.asarray(a_src1, np.float32))],
        axis=1).astype(BF)
    wad1 = _wcat(W1f, np.asarray(a_dst1, np.float32)).astype(BF)
    W2f = np.asarray(W2, np.float32)
    w2cat = np.concatenate(
        [W2f, _wcat(W2f, np.asarray(a_src2, np.float32))],
        axis=1).astype(BF)
    wad2 = _wcat(W2f, np.asarray(a_dst2, np.float32)).astype(BF)
    b1rep = np.tile(np.asarray(b1, np.float32)[None, :], (128, 1))
    b2rep = np.tile(np.asarray(b2, np.float32)[None, :], (128, 1))
    iota_bf = np.tile(np.arange(128, dtype=np.float32), (128, 1)).astype(BF)
    # fcW1 rows permuted to f-major g layout
    fcW1p = np.asarray(fcW1, np.float32)
    fcb1a = np.asarray(fcb1, np.float32).reshape(4, 128).T.copy()
    fcb2a = np.asarray(fcb2, np.float32).reshape(4, 128).T.copy()
    fw3a = np.asarray(fcW3, np.float32).reshape(4, 128).T.astype(BF).copy()

    in_maps = []
    for c in range(NCORES):
        xTown = np.ascontiguousarray(
            xpad[c * NSHP:(c + 1) * NSHP].T).astype(BF)
        in_maps.append(dict(
            xT=xT, xTown=xTown, w1cat=w1cat, wad1=wad1, b1rep=b1rep,
            w2cat=w2cat, wad2=wad2, b2rep=b2rep,
            idx_src=meta["idx_src"][c], idx_dst=meta["idx_dst"][c],
            dstl=meta["dstl"][c], iota_bf=iota_bf, glw=meta["glw"][c],
            gridx=meta["gridx"][c],
            fcw1=fcW1p.astype(BF),
            fcw2=np.asarray(fcW2, np.float32).astype(BF), fcw3=fw3a,
            fcb1=fcb1a, fcb2=fcb2a,
            fcb3=np.asarray(fcb3, np.float32).reshape(1, 1)))
    return in_maps


# ----------------------------------------------------------------- kernel()
def kernel(x, edge_index, batch, W1, a_src1, a_dst1, b1, W2, a_src2, a_dst2, b2,
           fcW1, fcb1, fcW2, fcb2, fcW3, fcb3):
    import jax

    raws = (x, edge_index, batch, W1, a_src1, a_dst1, b1, W2, a_src2, a_dst2,
            b2, fcW1, fcb1, fcW2, fcb2, fcW3, fcb3)
    raws = tuple(np.asarray(a) for a in raws)
    cached = _cache.get("raws")
    same = [cached is not None and len(cached) == len(raws)
            and a.shape == b.shape and np.array_equal(a, b)
            for a, b in zip(raws, cached or raws)]
    hit = bool(same) and all(same)

    if not hit:
        graph_same = bool(same) and same[1] and same[2] and "meta" in _cache
        _cache["raws"] = tuple(np.array(a, copy=True) for a in raws)
        meta = _cache["meta"] if graph_same else _preprocess(
            np.asarray(raws[1]), np.asarray(raws[2]))
        _cache["meta"] = meta
        key = (meta["TOTCH"], len(meta["gathers"]), tuple(meta["g0"]))
        if _cache.get("progkey") != key:
            _cache["prog"] = _build_merged(meta)
            _cache["progkey"] = key
            _cache.pop("runner", None)
            _cache.pop("benchrunner", None)
            _cache.pop("benchprog", None)

        in_maps = _host_prep(raws, meta)
        _cache["in_maps"] = in_maps
        st = _ensure_runner(_cache["prog"], "runner")
        st["dev_args"] = [
            jax.device_put(
                np.concatenate([np.asarray(m[name]) for m in in_maps], axis=0),
                st["shard"])
            for name in st["in_names"]]

    st = _cache["runner"]
    zeros = [jax.device_put(
        np.zeros((NCORES * av.shape[0], *av.shape[1:]), av.dtype), st["shard"])
        for av in st["out_avals"]]

    t0 = time.time()
    outs = st["fn"](*st["dev_args"], *zeros)
    res = [np.asarray(o) for o in outs]
    LAST_TIMES.clear()
    LAST_TIMES["p"] = time.time() - t0

    oi = st["out_names"].index("out")
    out = res[oi].reshape(NCORES, GSH).reshape(-1)  # [2048] in core order
    return out[:G].astype(np.float32).reshape(G, 1)


# ----------------------------------------------------------- bench (K iters)
BENCH_ITERS = 4


def bench_call():
    """Run the K-iteration program once; returns (wall_s, out[G,1]).

    Requires kernel() to have been called at least once (device inputs
    cached). The K-iteration NEFF executes the full kernel K times
    back-to-back on device, so wall = RTT + K * t_exec.
    """
    import jax
    if "benchprog" not in _cache:
        _cache["benchprog"] = _build_merged(_cache["meta"], iters=BENCH_ITERS)
    st = _ensure_runner(_cache["benchprog"], "benchrunner")
    if "dev_args" not in st:
        in_maps = _cache["in_maps"]
        st["dev_args"] = [
            jax.device_put(
                np.concatenate([np.asarray(m[name]) for m in in_maps], axis=0),
                st["shard"])
            for name in st["in_names"]]
    zeros = [jax.device_put(
        np.zeros((NCORES * av.shape[0], *av.shape[1:]), av.dtype), st["shard"])
        for av in st["out_avals"]]
    t0 = time.time()
    outs = st["fn"](*st["dev_args"], *zeros)
    res = [np.asarray(o) for o in outs]
    wall = time.time() - t0
    oi = st["out_names"].index("out")
    out = res[oi].reshape(NCORES, GSH).reshape(-1)
    return wall, out[:G].astype(np.float32).reshape(G, 1)


# revision 8
# speedup vs baseline: 17.5238x; 16.2481x over previous
"""GAT (2-layer, 8-head) + graph pooling + MLP on 8 TRN2 NeuronCores.

Single merged SPMD program (one dispatch per call) with on-device collectives:
  - layer-1 gather table is REPLICATED: every core computes all 50176 rows
    from the full x (cheap [32,520] matmuls) -> no AllGather-1
  - layer-2: AllGather only the bf16 h1^T [64, 6272] per core (0.8MB), then
    every core computes the full layer-2 table locally
  - tables use f-major head layout (row = [f0h0..f0h7, f1h0..]) so the
    per-edge msg multiply qualifies for DVE 2x_1P mode
  - per layer: edge pass with dma_gather by src (640-wide bf16 rows),
    attention ex = exp(leakyrelu(as+ad)), one-hot S matmuls for
    segment-softmax sum/den per 128-dst block
  - layer-2 epilogue pools nodes into 3 local 128-graph windows, windows
    land in a [2048, 512] global graph grid via dma_gather, then a
    ReduceScatter hands core c the pooled rows [256c, 256c+256)
  - each core runs the small MLP on its 256 graphs -> out [1, 256]

`_build_merged(meta, iters=K)` unrolls the whole kernel K times inside one
NEFF (same buffers) so test.py can subtract the host-dispatch RTT:
t_exec = (wall(K) - wall(1)) / (K - 1).
"""
import os
import time
import numpy as np
import ml_dtypes
from contextlib import ExitStack

os.environ.setdefault("JAX_PLATFORMS", "")  # allow axon platform auto-detect

LAST_TIMES = {}

import concourse.bass as bass
import concourse.bacc as bacc
import concourse.mybir as mybir
import concourse.tile as tile
from concourse.masks import make_identity

BF = ml_dtypes.bfloat16

N = 50000
E = 800000
NF = 32
H = 8
F = 64
HF = 512
G = 2000
NEG = 0.2
NCORES = 8
NSH = N // NCORES            # 6250 dst nodes per core
NBLK = (NSH + 127) // 128    # 49 blocks
NSHP = NBLK * 128            # 6272
NPADN = NCORES * NSHP        # 50176 padded-global rows (core-major)
HALF = 4 * NSHP              # 25088 = cores 0-3 -> table A (int16 idx range)
NWIN = 3                     # 128-graph windows per core (local range < 384)
WROWS = 512                  # winbuf rows: 384 window rows + zero pad
ZROW = 400                   # a guaranteed-zero winbuf row for out-of-range
GRID = 2048                  # global graph grid rows (>= G, 16 tiles of 128)
GSH = GRID // NCORES         # 256 graphs per core after ReduceScatter

F32 = mybir.dt.float32
BF16 = mybir.dt.bfloat16
I16 = mybir.dt.int16

_cache = {}


# ---------------------------------------------------------------- host prep
def _preprocess(edge_index, batch):
    src = np.concatenate([edge_index[0], np.arange(N, dtype=np.int64)]).astype(np.int64)
    dst = np.concatenate([edge_index[1], np.arange(N, dtype=np.int64)]).astype(np.int64)
    core = dst // NSH
    dloc = (dst - core * NSH).astype(np.int64)
    blk = dloc // 128
    # padded-global table row of the source node (core-major, stride NSHP)
    spad = (src // NSH) * NSHP + (src % NSH)
    tab = (spad >= HALF).astype(np.int64)

    # group key per edge: (core, blk, tab)
    key = (core * NBLK + blk) * 2 + tab
    order = np.argsort(key, kind="stable")
    src_s, dst_s, key_s = spad[order], dst[order], key[order]
    counts = np.bincount(key_s, minlength=NCORES * NBLK * 2).reshape(NCORES, NBLK * 2)

    # uniform chunk counts across cores
    K = np.ceil(counts.max(axis=0) / 128.0).astype(np.int64)  # [NBLK*2]
    TOTCH = int(K.sum())
    choff = np.concatenate([[0], np.cumsum(K)])  # chunk offset per group

    # per-core flat edge slot arrays [TOTCH*128]
    srci = np.zeros((NCORES, TOTCH * 128), np.int16)
    dsti = np.zeros((NCORES, TOTCH * 128), np.int16)
    dstl = np.full((NCORES, TOTCH * 128), -1.0, np.float32)

    gstart = np.concatenate([[0], np.cumsum(counts.reshape(-1))[:-1]])
    gs = gstart.reshape(NCORES, NBLK * 2)
    for c in range(NCORES):
        for g in range(NBLK * 2):
            n = counts[c, g]
            if n == 0:
                continue
            s0 = gs[c, g]
            es, ed = src_s[s0:s0 + n], dst_s[s0:s0 + n]
            o0 = choff[g] * 128
            t = g & 1
            srci[c, o0:o0 + n] = (es - t * HALF).astype(np.int16)
            dl = (ed - c * NSH).astype(np.int64)
            dsti[c, o0:o0 + n] = dl.astype(np.int16)
            dstl[c, o0:o0 + n] = (dl - (g // 2) * 128).astype(np.float32)

    # gather runs: per group, runs of <=8 chunks
    gathers = []  # (tab, chunk0, nch)
    for g in range(NBLK * 2):
        k = int(K[g])
        c0 = int(choff[g])
        while k > 0:
            nch = min(k, 8)
            gathers.append((g & 1, c0, nch))
            c0 += nch
            k -= nch

    def wrap16(v):  # [n] -> [128, n//16] column-major wrap, replicated
        n = v.shape[0]
        return np.tile(v.reshape(n // 16, 16).T, (8, 1)).astype(np.int16)

    idx_src = [np.concatenate(
        [wrap16(srci[c, c0 * 128:(c0 + nch) * 128]) for (_, c0, nch) in gathers], axis=1)
        for c in range(NCORES)]
    idx_dst = [np.concatenate(
        [wrap16(dsti[c, c0 * 128:(c0 + nch) * 128]) for (_, c0, nch) in gathers], axis=1)
        for c in range(NCORES)]
    dstl_t = [dstl[c].reshape(TOTCH, 128).T.copy() for c in range(NCORES)]

    # block boundaries in chunk space: block b covers chunks [choff[2b], choff[2b+2])
    blk_first = [int(choff[2 * b]) for b in range(NBLK)]
    blk_last = [int(choff[2 * b + 2]) - 1 for b in range(NBLK)]

    # graph-local window values per (lane, block, window)
    g0 = [int(batch[c * NSH]) for c in range(NCORES)]
    glw = []
    for c in range(NCORES):
        gmax = int(batch[(c + 1) * NSH - 1]) if c < NCORES - 1 else int(batch[N - 1])
        assert gmax - g0[c] < NWIN * 128, "graph-local id range exceeds windows"
        a = np.full((128, NBLK * NWIN), -1.0, np.float32)
        for b in range(NBLK):
            nn = min(128, NSH - b * 128)
            nodes = c * NSH + b * 128 + np.arange(nn)
            gl = batch[nodes] - g0[c]
            for w in range(NWIN):
                a[:nn, b * NWIN + w] = gl - 128 * w
        glw.append(a)

    # grid gather indices: grid row r <- winbuf row (r - g0) if in window
    # range else the zeroed row ZROW
    gridx = []
    for c in range(NCORES):
        r = np.arange(GRID, dtype=np.int64) - g0[c]
        idx = np.where((r >= 0) & (r < NWIN * 128), r, ZROW).astype(np.int16)
        gridx.append(np.concatenate(
            [np.tile(idx[t * 128:(t + 1) * 128].reshape(8, 16).T, (8, 1))
             for t in range(GRID // 128)], axis=1))

    return dict(TOTCH=TOTCH, gathers=gathers, blk_first=blk_first, blk_last=blk_last,
                idx_src=idx_src, idx_dst=idx_dst, dstl=dstl_t, glw=glw, g0=g0,
                gridx=gridx)


def _wcat(Wmat, a_vec):
    # [fin, H*F] weight + per-head attention vec -> [fin, H] alpha weight
    fin = Wmat.shape[0]
    Wr = Wmat.reshape(fin, H, F)
    return np.einsum("fhk,hk->fh", Wr, a_vec)


def _fmajor(Wmat):
    # [fin, H*F] h-major columns -> f-major columns [fin, F*H]
    fin = Wmat.shape[0]
    return Wmat.reshape(fin, H, F).transpose(0, 2, 1).reshape(fin, H * F)


# ------------------------------------------------------------- device build
def _edge_pass(nc, tc, ctx, meta, tabA, tabB, adtab, tagb, consts, epil):
    """Shared edge-processing pass. epil(b, num_ps, den_ps) emits the block
    epilogue after the block's last chunk. f-major head layout throughout."""
    IC_off = 0
    sb = ctx.enter_context(tc.tile_pool(name=f"eg{tagb}", bufs=3))
    sbm = ctx.enter_context(tc.tile_pool(name=f"em{tagb}", bufs=6))
    psN = ctx.enter_context(tc.tile_pool(name=f"pn{tagb}", bufs=2, space="PSUM"))
    psD = ctx.enter_context(tc.tile_pool(name=f"pd{tagb}", bufs=2, space="PSUM"))

    iota_bf = consts["iota_bf"]
    dstl_sb = consts["dstl_sb"]
    isrc_sb = consts["isrc_sb"]
    idst_sb = consts["idst_sb"]

    num_ps = den_ps = None
    cur_blk = -1
    for (t, c0, nch) in meta["gathers"]:
        n = nch * 128
        cols = nch * 8
        gt = sb.tile([128, nch, 640], BF16, tag="maing")
        nc.gpsimd.dma_gather(
            out_ap=gt[:], in_ap=(tabA if t == 0 else tabB),
            idxs_ap=isrc_sb[:, IC_off:IC_off + cols],
            num_idxs=n, num_idxs_reg=n, elem_size=640)
        adt = sb.tile([128, nch, 128], BF16, tag="adg")
        nc.gpsimd.dma_gather(
            out_ap=adt[:], in_ap=adtab[:],
            idxs_ap=idst_sb[:, IC_off:IC_off + cols],
            num_idxs=n, num_idxs_reg=n, elem_size=128)
        IC_off += cols

        e_st = sb.tile([128, nch, 8], F32, tag="est")
        nc.vector.tensor_tensor(
            out=e_st[:], in0=gt[:, :, 512:520], in1=adt[:, :, 0:8],
            op=mybir.AluOpType.add)
        e_fl = e_st[:].rearrange("p a b -> p (a b)")
        t_sc = sb.tile([128, nch * 8], F32, tag="esc")
        nc.vector.tensor_scalar(out=t_sc[:], in0=e_fl, scalar1=NEG, scalar2=None,
                                op0=mybir.AluOpType.mult)
        nc.vector.tensor_tensor(out=e_fl, in0=e_fl, in1=t_sc[:],
                                op=mybir.AluOpType.max)
        ex_st = sb.tile([128, nch * 8], BF16, tag="exs")
        nc.scalar.activation(ex_st[:], e_fl, mybir.ActivationFunctionType.Exp)

        for j in range(nch):
            ch = c0 + j
            if num_ps is None or ch > meta["blk_last"][cur_blk]:
                cur_blk += 1
                num_ps = psN.tile([128, 512], F32, tag="nps")
                den_ps = psD.tile([128, 8], F32, tag="dps")
            S = sbm.tile([128, 128], BF16, tag="S")
            nc.vector.tensor_scalar(
                out=S[:], in0=iota_bf[:], scalar1=dstl_sb[:, ch:ch + 1],
                scalar2=None, op0=mybir.AluOpType.is_equal)
            msg = sbm.tile([128, 512], BF16, tag="msg")
            nc.vector.tensor_tensor(
                out=msg[:].rearrange("p (h f) -> p h f", h=H),
                in0=gt[:, j, 0:512].rearrange("p (h f) -> p h f", h=H),
                in1=ex_st[:, 8 * j:8 * j + 8].unsqueeze(2).to_broadcast([128, H, F]),
                op=mybir.AluOpType.mult)
            first = ch == meta["blk_first"][cur_blk]
            last = ch == meta["blk_last"][cur_blk]
            nc.tensor.matmul(num_ps[:], lhsT=S[:], rhs=msg[:], start=first, stop=last)
            nc.tensor.matmul(den_ps[:], lhsT=S[:], rhs=ex_st[:, 8 * j:8 * j + 8],
                             start=first, stop=last)
            if last:
                epil(cur_blk, num_ps, den_ps)


def _load_edge_consts(nc, tc, ctx, meta, inp, tagb):
    consts = {}
    cp = ctx.enter_context(tc.tile_pool(name=f"ec{tagb}", bufs=1))
    IC = sum(nch * 8 for (_, _, nch) in meta["gathers"])
    isrc_sb = cp.tile([128, IC], I16)
    nc.sync.dma_start(isrc_sb[:], inp["idx_src"][:])
    idst_sb = cp.tile([128, IC], I16)
    nc.sync.dma_start(idst_sb[:], inp["idx_dst"][:])
    dstl_sb = cp.tile([128, meta["TOTCH"]], F32)
    nc.sync.dma_start(dstl_sb[:], inp["dstl"][:])
    iota_bf = cp.tile([128, 128], BF16)
    nc.sync.dma_start(iota_bf[:], inp["iota_bf"][:])
    consts.update(isrc_sb=isrc_sb, idst_sb=idst_sb, dstl_sb=dstl_sb, iota_bf=iota_bf)
    return consts


def _emit_iter(nc, tc, ctx, meta, io, groups, it):
    """Emit one full kernel iteration. io: dict of dram tensors."""
    tg = f"_{it}"
    cp = ctx.enter_context(tc.tile_pool(name=f"wc{tg}", bufs=1))
    w1_sb = cp.tile([32, 520], BF16)
    nc.sync.dma_start(w1_sb[:], io["i_w1"][:])
    wad1_sb = cp.tile([32, 8], BF16)
    nc.sync.dma_start(wad1_sb[:], io["i_wad1"][:])
    b1_sb = cp.tile([128, 64], F32)
    nc.sync.dma_start(b1_sb[:], io["i_b1"][:])
    w2_sb = cp.tile([64, 520], BF16)
    nc.sync.dma_start(w2_sb[:], io["i_w2"][:])
    wad2_sb = cp.tile([64, 8], BF16)
    nc.sync.dma_start(wad2_sb[:], io["i_wad2"][:])
    b2_sb = cp.tile([128, 512], F32)
    nc.sync.dma_start(b2_sb[:], io["i_b2"][:])
    glw_sb = cp.tile([128, NBLK * NWIN], F32)
    nc.sync.dma_start(glw_sb[:], io["i_glw"][:])
    gridx_sb = cp.tile([128, GRID // 16], I16)
    nc.sync.dma_start(gridx_sb[:], io["i_gridx"][:])
    ident_bf = cp.tile([128, 128], BF16)
    make_identity(nc, ident_bf[:])
    xo_sb = cp.tile([32, NSHP], BF16)
    nc.sync.dma_start(xo_sb[:], io["i_xTown"][:])
    h1own = cp.tile([128, NBLK * 64], BF16)
    consts = _load_edge_consts(nc, tc, ctx, meta, dict(
        idx_src=io["i_isrc"], idx_dst=io["i_idst"], dstl=io["i_dstl"],
        iota_bf=io["i_iota"]), tg)

    tab1, adtab1 = io["tab1"], io["adtab1"]
    tab2, adtab2 = io["tab2"], io["adtab2"]
    h1locT, h1Tfull = io["h1locT"], io["h1Tfull"]
    winbuf, grid, gmine = io["winbuf"], io["grid"], io["gmine"]

    # ---------------- layer-1 adtab (own shard) + replicated full table
    with ExitStack() as tctx:
        psA = tctx.enter_context(tc.tile_pool(name=f"t1pa{tg}", bufs=2, space="PSUM"))
        rowp = tctx.enter_context(tc.tile_pool(name=f"t1row{tg}", bufs=3))
        for b in range(NBLK):
            dps = psA.tile([128, 8], F32, tag="dps")
            nc.tensor.matmul(dps[:], lhsT=xo_sb[:, b * 128:(b + 1) * 128],
                             rhs=wad1_sb[:, 0:8], start=True, stop=True)
            adrow = rowp.tile([128, 128], BF16, tag="adrow")
            nc.vector.memset(adrow[:, 8:128], 0.0)
            nc.vector.tensor_copy(adrow[:, 0:8], dps[:])
            nc.sync.dma_start(adtab1[b * 128:(b + 1) * 128, :], adrow[:])

    with ExitStack() as tctx:
        ps5 = tctx.enter_context(tc.tile_pool(name=f"t1p5{tg}", bufs=2, space="PSUM"))
        ps8 = tctx.enter_context(tc.tile_pool(name=f"t1p8{tg}", bufs=2, space="PSUM"))
        rowp = tctx.enter_context(tc.tile_pool(name=f"t1row{tg}b", bufs=4))
        xsp = tctx.enter_context(tc.tile_pool(name=f"t1xs{tg}", bufs=2))
        for cs in range(NCORES):
            xs = xsp.tile([32, NSHP], BF16, tag="xs")
            nc.sync.dma_start(xs[:], io["i_xT"][:, cs * NSHP:(cs + 1) * NSHP])
            for b in range(NBLK):
                lhsT = xs[:, b * 128:(b + 1) * 128]
                hps = ps5.tile([128, 512], F32, tag="hps")
                nc.tensor.matmul(hps[:], lhsT=lhsT, rhs=w1_sb[:, 0:512],
                                 start=True, stop=True)
                aps = ps8.tile([128, 8], F32, tag="aps")
                nc.tensor.matmul(aps[:], lhsT=lhsT, rhs=w1_sb[:, 512:520],
                                 start=True, stop=True)
                row = rowp.tile([128, 640], BF16, tag="row")
                if b % 2 == 0:
                    nc.scalar.copy(row[:, 0:512], hps[:])
                else:
                    nc.vector.tensor_copy(row[:, 0:512], hps[:])
                nc.vector.tensor_copy(row[:, 512:520], aps[:])
                r0 = cs * NSHP + b * 128
                nc.sync.dma_start(tab1[r0:r0 + 128, 0:520], row[:, 0:520])

    # ---------------- layer-1 edge pass -> h1own + h1locT + adtab2
    with ExitStack() as ectx:
        ep = ectx.enter_context(tc.tile_pool(name=f"ep1{tg}", bufs=3))
        psT = ectx.enter_context(tc.tile_pool(name=f"e1pt{tg}", bufs=2, space="PSUM"))
        psA = ectx.enter_context(tc.tile_pool(name=f"e1pa{tg}", bufs=2, space="PSUM"))
        htp = ectx.enter_context(tc.tile_pool(name=f"e1ht{tg}", bufs=2))

        def epil1(b, num_ps, den_ps):
            den = ep.tile([128, 8], F32, tag="den")
            nc.vector.tensor_scalar(out=den[:], in0=den_ps[:], scalar1=8.0,
                                    scalar2=1e-20, op0=mybir.AluOpType.mult,
                                    op1=mybir.AluOpType.add)
            rec = ep.tile([128, 8], F32, tag="rec")
            nc.vector.reciprocal(rec[:], den[:])
            tmp = ep.tile([128, 512], F32, tag="tmp")
            nc.vector.tensor_tensor(
                out=tmp[:].rearrange("p (h f) -> p h f", h=H),
                in0=num_ps[:].rearrange("p (h f) -> p h f", h=H),
                in1=rec[:].unsqueeze(2).to_broadcast([128, H, F]),
                op=mybir.AluOpType.mult)
            t3 = tmp[:].rearrange("p (h f) -> p h f", h=H)
            a4 = ep.tile([128, 256], F32, tag="a4")
            nc.vector.tensor_tensor(
                out=a4[:].rearrange("p (h f) -> p h f", h=4),
                in0=t3[:, 0:4, :], in1=t3[:, 4:8, :], op=mybir.AluOpType.add)
            a4v = a4[:].rearrange("p (h f) -> p h f", h=4)
            a2 = ep.tile([128, 128], F32, tag="a2")
            nc.vector.tensor_tensor(
                out=a2[:].rearrange("p (h f) -> p h f", h=2),
                in0=a4v[:, 0:2, :], in1=a4v[:, 2:4, :], op=mybir.AluOpType.add)
            a2v = a2[:].rearrange("p (h f) -> p h f", h=2)
            a1 = ep.tile([128, 64], F32, tag="a1")
            nc.vector.tensor_tensor(out=a1[:], in0=a2v[:, 0, :], in1=a2v[:, 1, :],
                                    op=mybir.AluOpType.add)
            nc.vector.tensor_tensor(out=h1own[:, b * 64:(b + 1) * 64],
                                    in0=a1[:], in1=b1_sb[:],
                                    op=mybir.AluOpType.add)
            # transpose h1 block -> h1locT column block + layer-2 adtab row
            tps = psT.tile([64, 128], BF16, tag="tps")
            nc.tensor.transpose(tps[:], h1own[:, b * 64:(b + 1) * 64], ident_bf[:])
            hT = htp.tile([64, 128], BF16, tag="hT")
            nc.scalar.copy(hT[:], tps[:])
            nc.sync.dma_start(h1locT[:, b * 128:(b + 1) * 128], hT[:])
            dps = psA.tile([128, 8], F32, tag="dps")
            nc.tensor.matmul(dps[:], lhsT=hT[:], rhs=wad2_sb[:, 0:8],
                             start=True, stop=True)
            adrow = ep.tile([128, 128], BF16, tag="adrow")
            nc.vector.memset(adrow[:, 8:128], 0.0)
            nc.vector.tensor_copy(adrow[:, 0:8], dps[:])
            nc.sync.dma_start(adtab2[b * 128:(b + 1) * 128, :], adrow[:])

        _edge_pass(nc, tc, ectx, meta, tab1[0:HALF, :],
                   tab1[HALF:NPADN, :], adtab1, f"1{tg}", consts, epil1)

    # ---------------- AllGather h1^T (bf16, 0.8MB per core)
    nc.gpsimd.collective_compute(
        "AllGather", mybir.AluOpType.bypass, replica_groups=groups,
        ins=[h1locT[:].opt()], outs=[h1Tfull[:].opt()])

    # ---------------- layer-2 replicated full table
    with ExitStack() as tctx:
        ps5 = tctx.enter_context(tc.tile_pool(name=f"t2p5{tg}", bufs=2, space="PSUM"))
        ps8 = tctx.enter_context(tc.tile_pool(name=f"t2p8{tg}", bufs=2, space="PSUM"))
        rowp = tctx.enter_context(tc.tile_pool(name=f"t2row{tg}", bufs=4))
        hsp = tctx.enter_context(tc.tile_pool(name=f"t2hs{tg}", bufs=2))
        for cs in range(NCORES):
            hs = hsp.tile([64, NSHP], BF16, tag="hs")
            nc.sync.dma_start(hs[:], h1Tfull[cs * 64:(cs + 1) * 64, :])
            for b in range(NBLK):
                lhsT = hs[:, b * 128:(b + 1) * 128]
                hps = ps5.tile([128, 512], F32, tag="hps")
                nc.tensor.matmul(hps[:], lhsT=lhsT, rhs=w2_sb[:, 0:512],
                                 start=True, stop=True)
                aps = ps8.tile([128, 8], F32, tag="aps")
                nc.tensor.matmul(aps[:], lhsT=lhsT, rhs=w2_sb[:, 512:520],
                                 start=True, stop=True)
                row = rowp.tile([128, 640], BF16, tag="row")
                if b % 2 == 0:
                    nc.scalar.copy(row[:, 0:512], hps[:])
                else:
                    nc.vector.tensor_copy(row[:, 0:512], hps[:])
                nc.vector.tensor_copy(row[:, 512:520], aps[:])
                r0 = cs * NSHP + b * 128
                nc.sync.dma_start(tab2[r0:r0 + 128, 0:520], row[:, 0:520])

    # ---------------- layer-2 edge pass + window pooling
    with ExitStack() as ectx:
        ep = ectx.enter_context(tc.tile_pool(name=f"ep2{tg}", bufs=3))
        sgp = ectx.enter_context(tc.tile_pool(name=f"sg{tg}", bufs=3))
        psG = ectx.enter_context(tc.tile_pool(name=f"psg{tg}", bufs=1, space="PSUM"))
        gw_ps = []
        for w in range(NWIN):
            gw_tile = psG.tile([128, 512], F32, tag=f"gw{w}")
            gw_ps.append(gw_tile)

        def epil2(b, num_ps, den_ps):
            den = ep.tile([128, 8], F32, tag="den")
            nc.vector.tensor_scalar(out=den[:], in0=den_ps[:], scalar1=1e-20,
                                    scalar2=None, op0=mybir.AluOpType.add)
            rec = ep.tile([128, 8], F32, tag="rec")
            nc.vector.reciprocal(rec[:], den[:])
            o2f = ep.tile([128, 512], F32, tag="o2f")
            nc.vector.tensor_tensor(
                out=o2f[:].rearrange("p (h f) -> p h f", h=H),
                in0=num_ps[:].rearrange("p (h f) -> p h f", h=H),
                in1=rec[:].unsqueeze(2).to_broadcast([128, H, F]),
                op=mybir.AluOpType.mult)
            o2 = ep.tile([128, 512], BF16, tag="o2")
            nc.vector.tensor_tensor(out=o2[:], in0=o2f[:], in1=b2_sb[:],
                                    op=mybir.AluOpType.add)
            for w in range(NWIN):
                Sg = sgp.tile([128, 128], BF16, tag="Sg")
                nc.vector.tensor_scalar(
                    out=Sg[:], in0=consts["iota_bf"][:],
                    scalar1=glw_sb[:, b * NWIN + w:b * NWIN + w + 1],
                    scalar2=None, op0=mybir.AluOpType.is_equal)
                nc.tensor.matmul(gw_ps[w][:], lhsT=Sg[:], rhs=o2[:],
                                 start=(b == 0), stop=(b == NBLK - 1))

        _edge_pass(nc, tc, ectx, meta, tab2[0:HALF, :],
                   tab2[HALF:NPADN, :], adtab2, f"2{tg}", consts, epil2)

        # windows -> winbuf rows [0, 384); zero rows [384, 512)
        zt = ep.tile([128, 512], F32, tag="zt")
        nc.gpsimd.memset(zt[:], 0.0)
        nc.sync.dma_start(winbuf[NWIN * 128:WROWS, :], zt[:])
        for w in range(NWIN):
            wsb = ep.tile([128, 512], F32, tag="wsb")
            nc.vector.tensor_copy(wsb[:], gw_ps[w][:])
            nc.sync.dma_start(winbuf[w * 128:(w + 1) * 128, :], wsb[:])

    # ---------------- grid assembly + ReduceScatter
    with ExitStack() as gctx:
        gp = gctx.enter_context(tc.tile_pool(name=f"gridp{tg}", bufs=3))
        for t in range(GRID // 128):
            gtile = gp.tile([128, 1, 512], F32, tag="gtile")
            nc.gpsimd.dma_gather(
                out_ap=gtile[:], in_ap=winbuf[:],
                idxs_ap=gridx_sb[:, t * 8:(t + 1) * 8],
                num_idxs=128, num_idxs_reg=128, elem_size=512)
            nc.sync.dma_start(grid[t * 128:(t + 1) * 128, :],
                              gtile[:, 0, :])

    nc.gpsimd.collective_compute(
        "ReduceScatter", mybir.AluOpType.add, replica_groups=groups,
        ins=[grid[:].opt()], outs=[gmine[:].opt()])

    # ---------------- MLP on own 256 graphs
    with ExitStack() as mctx:
        mw = mctx.enter_context(tc.tile_pool(name=f"mw{tg}", bufs=1))
        fw1, fw2 = [], []
        for k in range(4):
            fw1_t = mw.tile([128, 512], BF16, tag=f"fw1{k}")
            fw1.append(fw1_t)
            fw2_t = mw.tile([128, 512], BF16, tag=f"fw2{k}")
            fw2.append(fw2_t)
        for k in range(4):
            nc.sync.dma_start(fw1[k][:], io["i_fw1"][k * 128:(k + 1) * 128, :])
            nc.sync.dma_start(fw2[k][:], io["i_fw2"][k * 128:(k + 1) * 128, :])
        fw3 = mw.tile([128, 4], BF16)
        nc.sync.dma_start(fw3[:], io["i_fw3"][:])
        fb1 = mw.tile([128, 4], F32)
        nc.sync.dma_start(fb1[:], io["i_fb1"][:])
        fb2 = mw.tile([128, 4], F32)
        nc.sync.dma_start(fb2[:], io["i_fb2"][:])
        fb3 = mw.tile([1, 1], F32)
        nc.sync.dma_start(fb3[:], io["i_fb3"][:])
        ident_f = mw.tile([128, 128], F32)
        make_identity(nc, ident_f[:])

        gp = mctx.enter_context(tc.tile_pool(name=f"mg{tg}", bufs=2))
        psT = mctx.enter_context(tc.tile_pool(name=f"mpt{tg}", bufs=2, space="PSUM"))
        psA = mctx.enter_context(tc.tile_pool(name=f"mpa{tg}", bufs=2, space="PSUM"))
        psO = mctx.enter_context(tc.tile_pool(name=f"mpo{tg}", bufs=2, space="PSUM"))
        ap_ = mctx.enter_context(tc.tile_pool(name=f"ma{tg}", bufs=2))

        for gt in range(GSH // 128):
            gl = gp.tile([128, 512], F32, tag="gl")
            nc.sync.dma_start(gl[:], gmine[gt * 128:(gt + 1) * 128, :])
            gTs = []
            for k in range(4):
                tps = psT.tile([128, 128], F32, tag="tps")
                nc.tensor.transpose(tps[:], gl[:, k * 128:(k + 1) * 128],
                                    ident_f[:])
                gT = ap_.tile([128, 128], BF16, tag=f"gT{k}")
                nc.vector.tensor_copy(gT[:], tps[:])
                gTs.append(gT)
            a1s, a2s = [], []
            for m in range(4):
                aps = psA.tile([128, 128], F32, tag="aps")
                for k in range(4):
                    nc.tensor.matmul(aps[:], lhsT=fw1[k][:, m * 128:(m + 1) * 128],
                                     rhs=gTs[k][:], start=(k == 0), stop=(k == 3))
                a1 = ap_.tile([128, 128], BF16, tag=f"a1{m}")
                nc.scalar.activation(a1[:], aps[:],
                                     mybir.ActivationFunctionType.Relu,
                                     bias=fb1[:, m:m + 1])
                a1s.append(a1)
            for m in range(4):
                aps = psA.tile([128, 128], F32, tag="bps")
                for k in range(4):
                    nc.tensor.matmul(aps[:], lhsT=fw2[k][:, m * 128:(m + 1) * 128],
                                     rhs=a1s[k][:], start=(k == 0), stop=(k == 3))
                a2 = ap_.tile([128, 128], BF16, tag=f"a2{m}")
                nc.scalar.activation(a2[:], aps[:],
                                     mybir.ActivationFunctionType.Relu,
                                     bias=fb2[:, m:m + 1])
                a2s.append(a2)
            ops = psO.tile([128, 128], F32, tag="ops")
            for k in range(4):
                nc.tensor.matmul(ops[0:1, :], lhsT=fw3[:, k:k + 1], rhs=a2s[k][:],
                                 start=(k == 0), stop=(k == 3))
            osb = ap_.tile([128, 128], F32, tag="osb")
            nc.scalar.activation(osb[0:1, :], ops[0:1, :],
                                 mybir.ActivationFunctionType.Identity,
                                 bias=fb3[0:1, 0:1])
            nc.sync.dma_start(io["o_out"][0:1, gt * 128:(gt + 1) * 128], osb[0:1, :])


def _build_merged(meta, iters=1):
    nc = bacc.Bacc("TRN2", target_bir_lowering=False, debug=False, num_devices=NCORES)
    IC = sum(nch * 8 for (_, _, nch) in meta["gathers"])
    groups = [list(range(NCORES))]

    io = {}
    io["i_xT"] = nc.dram_tensor("xT", [32, NPADN], BF16, kind="ExternalInput")
    io["i_xTown"] = nc.dram_tensor("xTown", [32, NSHP], BF16, kind="ExternalInput")
    io["i_w1"] = nc.dram_tensor("w1cat", [32, 520], BF16, kind="ExternalInput")
    io["i_wad1"] = nc.dram_tensor("wad1", [32, 8], BF16, kind="ExternalInput")
    io["i_b1"] = nc.dram_tensor("b1rep", [128, 64], F32, kind="ExternalInput")
    io["i_w2"] = nc.dram_tensor("w2cat", [64, 520], BF16, kind="ExternalInput")
    io["i_wad2"] = nc.dram_tensor("wad2", [64, 8], BF16, kind="ExternalInput")
    io["i_b2"] = nc.dram_tensor("b2rep", [128, 512], F32, kind="ExternalInput")
    io["i_isrc"] = nc.dram_tensor("idx_src", [128, IC], I16, kind="ExternalInput")
    io["i_idst"] = nc.dram_tensor("idx_dst", [128, IC], I16, kind="ExternalInput")
    io["i_dstl"] = nc.dram_tensor("dstl", [128, meta["TOTCH"]], F32, kind="ExternalInput")
    io["i_iota"] = nc.dram_tensor("iota_bf", [128, 128], BF16, kind="ExternalInput")
    io["i_glw"] = nc.dram_tensor("glw", [128, NBLK * NWIN], F32, kind="ExternalInput")
    io["i_gridx"] = nc.dram_tensor("gridx", [128, GRID // 16], I16, kind="ExternalInput")
    io["i_fw1"] = nc.dram_tensor("fcw1", [512, 512], BF16, kind="ExternalInput")
    io["i_fw2"] = nc.dram_tensor("fcw2", [512, 512], BF16, kind="ExternalInput")
    io["i_fw3"] = nc.dram_tensor("fcw3", [128, 4], BF16, kind="ExternalInput")
    io["i_fb1"] = nc.dram_tensor("fcb1", [128, 4], F32, kind="ExternalInput")
    io["i_fb2"] = nc.dram_tensor("fcb2", [128, 4], F32, kind="ExternalInput")
    io["i_fb3"] = nc.dram_tensor("fcb3", [1, 1], F32, kind="ExternalInput")
    io["o_out"] = nc.dram_tensor("out", [1, GSH], F32, kind="ExternalOutput")

    with tile.TileContext(nc, num_cores=NCORES) as tc:
        with ExitStack() as ctx:
            dram = ctx.enter_context(tc.tile_pool(name="dram", bufs=1, space="DRAM"))
            io["tab1"] = dram.tile([NPADN, 640], BF16, name="tab1")
            io["adtab1"] = dram.tile([NSHP, 128], BF16, name="adtab1")
            io["tab2"] = dram.tile([NPADN, 640], BF16, name="tab2")
            io["adtab2"] = dram.tile([NSHP, 128], BF16, name="adtab2")
            io["h1locT"] = dram.tile([64, NSHP], BF16, name="h1locT")
            io["winbuf"] = dram.tile([WROWS, 512], F32, name="winbuf")
            io["grid"] = dram.tile([GRID, 512], F32, name="grid")

            for it in range(iters):
                # collective outputs: single-writer rule -> one tile per iter
                io2 = dict(io)
                io2["h1Tfull"] = dram.tile(
                    [NCORES * 64, NSHP], BF16, addr_space="Shared",
                    name=f"h1Tfull{it}")
                io2["gmine"] = dram.tile([GSH, 512], F32, name=f"gmine{it}")
                with ExitStack() as bctx:
                    _emit_iter(nc, tc, bctx, meta, io2, groups, it)

    nc.compile()
    return nc


# ------------------------------------------------- cached PJRT runner
def _ensure_runner(nc, key):
    """Build the jitted shard_map executor for nc (once per program)."""
    import jax
    from jax.sharding import Mesh, PartitionSpec, NamedSharding
    from jax.experimental.shard_map import shard_map
    from concourse import bass2jax
    from concourse.bass2jax import _bass_exec_p, partition_id_tensor

    st = _cache.setdefault(key, {})
    if "fn" in st:
        return st
    bass2jax.install_neuronx_cc_hook()
    partition_name = (nc.partition_id_tensor.name
                      if nc.partition_id_tensor else None)
    in_names, out_names, out_avals = [], [], []
    for alloc in nc.m.functions[0].allocations:
        if not isinstance(alloc, mybir.MemoryLocationSet):
            continue
        name = alloc.memorylocations[0].name
        if alloc.kind == "ExternalInput":
            if name != partition_name:
                in_names.append(name)
        elif alloc.kind == "ExternalOutput":
            shape = tuple(alloc.tensor_shape)
            dtype = mybir.dt.np(alloc.dtype)
            out_names.append(name)
            out_avals.append(jax.core.ShapedArray(shape, dtype))
    n_params = len(in_names)
    all_names = list(in_names) + list(out_names)
    if partition_name is not None:
        all_names.append(partition_name)
    donate = tuple(range(n_params, n_params + len(out_names)))

    def _body(*args):
        operands = list(args)
        if partition_name is not None:
            operands.append(partition_id_tensor())
        outs = _bass_exec_p.bind(
            *operands, out_avals=tuple(out_avals), in_names=tuple(all_names),
            out_names=tuple(out_names), lowering_input_output_aliases=(),
            sim_require_finite=True, sim_require_nnan=True, nc=nc)
        return tuple(outs)

    devices = jax.devices()[:NCORES]
    mesh = Mesh(np.asarray(devices), ("core",))
    spec_in = (PartitionSpec("core"),) * (n_params + len(out_names))
    spec_out = (PartitionSpec("core"),) * len(out_names)
    fn = jax.jit(shard_map(_body, mesh=mesh, in_specs=spec_in,
                           out_specs=spec_out, check_rep=False),
                 donate_argnums=donate, keep_unused=True)
    st.update(fn=fn, in_names=in_names, out_names=out_names,
              out_avals=out_avals,
              shard=NamedSharding(mesh, PartitionSpec("core")))
    return st


def _host_prep(raws, meta):
    (x, edge_index, batch, W1, a_src1, a_dst1, b1, W2, a_src2, a_dst2,
     b2, fcW1, fcb1, fcW2, fcb2, fcW3, fcb3) = raws
    x = np.asarray(x, np.float32)
    xpad = np.zeros((NPADN, NF), np.float32)
    for c in range(NCORES):
        xpad[c * NSHP:c * NSHP + NSH] = x[c * NSH:(c + 1) * NSH]
    xT = np.ascontiguousarray(xpad.T).astype(BF)
    W1f = np.asarray(W1, np.float32)
    w1cat = np.concatenate(
        [W1f, _wcat(W1f, np.asarray(a_src1, np.float32))],
        axis=1).astype(BF)
    wad1 = _wcat(W1f, np.asarray(a_dst1, np.float32)).astype(BF)
    W2f = np.asarray(W2, np.float32)
    w2cat = np.concatenate(
        [W2f, _wcat(W2f, np.asarray(a_src2, np.float32))],
        axis=1).astype(BF)
    wad2 = _wcat(W2f, np.asarray(a_dst2, np.float32)).astype(BF)
    b1rep = np.tile(np.asarray(b1, np.float32)[None, :], (128, 1))
    b2rep = np.tile(np.asarray(b2, np.float32)[None, :], (128, 1))
    iota_bf = np.tile(np.arange(128, dtype=np.float32), (128, 1)).astype(BF)
    # fcW1 rows permuted to f-major g layout
    fcW1p = np.asarray(fcW1, np.float32)
    fcb1a = np.asarray(fcb1, np.float32).reshape(4, 128).T.copy()
    fcb2a = np.asarray(fcb2, np.float32).reshape(4, 128).T.copy()
    fw3a = np.asarray(fcW3, np.float32).reshape(4, 128).T.astype(BF).copy()

    in_maps = []
    for c in range(NCORES):
        xTown = np.ascontiguousarray(
            xpad[c * NSHP:(c + 1) * NSHP].T).astype(BF)
        in_maps.append(dict(
            xT=xT, xTown=xTown, w1cat=w1cat, wad1=wad1, b1rep=b1rep,
            w2cat=w2cat, wad2=wad2, b2rep=b2rep,
            idx_src=meta["idx_src"][c], idx_dst=meta["idx_dst"][c],
            dstl=meta["dstl"][c], iota_bf=iota_bf, glw=meta["glw"][c],
            gridx=meta["gridx"][c],
            fcw1=fcW1p.astype(BF),
            fcw2=np.asarray(fcW2, np.float32).astype(BF), fcw3=fw3a,
            fcb1=fcb1a, fcb2=fcb2a,
            fcb3=np.asarray(fcb3, np.float32).reshape(1, 1)))
    return in_maps


# ----------------------------------------------------------------- kernel()
def kernel(x, edge_index, batch, W1, a_src1, a_dst1, b1, W2, a_src2, a_dst2, b2,
           fcW1, fcb1, fcW2, fcb2, fcW3, fcb3):
    import jax

    raws = (x, edge_index, batch, W1, a_src1, a_dst1, b1, W2, a_src2, a_dst2,
            b2, fcW1, fcb1, fcW2, fcb2, fcW3, fcb3)
    raws = tuple(np.asarray(a) for a in raws)
    cached = _cache.get("raws")
    same = [cached is not None and len(cached) == len(raws)
            and a.shape == b.shape and np.array_equal(a, b)
            for a, b in zip(raws, cached or raws)]
    hit = bool(same) and all(same)

    if not hit:
        graph_same = bool(same) and same[1] and same[2] and "meta" in _cache
        _cache["raws"] = tuple(np.array(a, copy=True) for a in raws)
        meta = _cache["meta"] if graph_same else _preprocess(
            np.asarray(raws[1]), np.asarray(raws[2]))
        _cache["meta"] = meta
        key = (meta["TOTCH"], len(meta["gathers"]), tuple(meta["g0"]))
        if _cache.get("progkey") != key:
            _cache["prog"] = _build_merged(meta)
            _cache["progkey"] = key
            _cache.pop("runner", None)
            _cache.pop("benchrunner", None)
            _cache.pop("benchprog", None)

        in_maps = _host_prep(raws, meta)
        _cache["in_maps"] = in_maps
        st = _ensure_runner(_cache["prog"], "runner")
        st["dev_args"] = [
            jax.device_put(
                np.concatenate([np.asarray(m[name]) for m in in_maps], axis=0),
                st["shard"])
            for name in st["in_names"]]

    st = _cache["runner"]
    zeros = [jax.device_put(
        np.zeros((NCORES * av.shape[0], *av.shape[1:]), av.dtype), st["shard"])
        for av in st["out_avals"]]

    t0 = time.time()
    outs = st["fn"](*st["dev_args"], *zeros)
    res = [np.asarray(o) for o in outs]
    LAST_TIMES.clear()
    LAST_TIMES["p"] = time.time() - t0

    oi = st["out_names"].index("out")
    out = res[oi].reshape(NCORES, GSH).reshape(-1)  # [2048] in core order
    return out[:G].astype(np.float32).reshape(G, 1)


# ----------------------------------------------------------- bench (K iters)
BENCH_ITERS = 16


def bench_call():
    """Run the K-iteration program once; returns (wall_s, out[G,1]).

    Requires kernel() to have been called at least once (device inputs
    cached). The K-iteration NEFF executes the full kernel K times
    back-to-back on device, so wall = RTT + K * t_exec.
    """
    import jax
    if "benchprog" not in _cache:
        _cache["benchprog"] = _build_merged(_cache["meta"], iters=BENCH_ITERS)
    st = _ensure_runner(_cache["benchprog"], "benchrunner")
    if "dev_args" not in st:
        in_maps = _cache["in_maps"]
        st["dev_args"] = [
            jax.device_put(
                np.concatenate([np.asarray(m[name]) for m in in_maps], axis=0),
                st["shard"])
            for name in st["in_names"]]
    zeros = [jax.device_put(
        np.zeros((NCORES * av.shape[0], *av.shape[1:]), av.dtype), st["shard"])
        for av in st["out_avals"]]
    t0 = time.time()
    outs = st["fn"](*st["dev_args"], *zeros)
    res = [np.asarray(o) for o in outs]
    wall = time.time() - t0
    oi = st["out_names"].index("out")
    out = res[oi].reshape(NCORES, GSH).reshape(-1)
    return wall, out[:G].astype(np.float32).reshape(G, 1)


# revision 9
# speedup vs baseline: 18.1187x; 1.0339x over previous
"""GAT (2-layer, 8-head) + graph pooling + MLP on 8 TRN2 NeuronCores.

Single merged SPMD program (one dispatch per call) with on-device collectives:
  - layer-1 gather table is REPLICATED: every core computes all 50176 rows
    from the full x (cheap [32,520] matmuls) -> no AllGather-1
  - layer-2: AllGather only the bf16 h1^T [64, 6272] per core (0.8MB), then
    every core computes the full layer-2 table locally
  - tables use f-major head layout (row = [f0h0..f0h7, f1h0..]) so the
    per-edge msg multiply qualifies for DVE 2x_1P mode
  - per layer: edge pass with dma_gather by src (640-wide bf16 rows),
    attention ex = exp(leakyrelu(as+ad)), one-hot S matmuls for
    segment-softmax sum/den per 128-dst block
  - layer-2 epilogue pools nodes into 3 local 128-graph windows, windows
    land in a [2048, 512] global graph grid via dma_gather, then a
    ReduceScatter hands core c the pooled rows [256c, 256c+256)
  - each core runs the small MLP on its 256 graphs -> out [1, 256]

`_build_merged(meta, iters=K)` unrolls the whole kernel K times inside one
NEFF (same buffers) so test.py can subtract the host-dispatch RTT:
t_exec = (wall(K) - wall(1)) / (K - 1).
"""
import os
import time
import numpy as np
import ml_dtypes
from contextlib import ExitStack

os.environ.setdefault("JAX_PLATFORMS", "")  # allow axon platform auto-detect

LAST_TIMES = {}

import concourse.bass as bass
import concourse.bacc as bacc
import concourse.mybir as mybir
import concourse.tile as tile
from concourse.masks import make_identity

BF = ml_dtypes.bfloat16

N = 50000
E = 800000
NF = 32
H = 8
F = 64
HF = 512
G = 2000
NEG = 0.2
NCORES = 8
NSH = N // NCORES            # 6250 dst nodes per core
NBLK = (NSH + 127) // 128    # 49 blocks
NSHP = NBLK * 128            # 6272
NPADN = NCORES * NSHP        # 50176 padded-global rows (core-major)
HALF = 4 * NSHP              # 25088 = cores 0-3 -> table A (int16 idx range)
NWIN = 3                     # 128-graph windows per core (local range < 384)
WROWS = 512                  # winbuf rows: 384 window rows + zero pad
ZROW = 400                   # a guaranteed-zero winbuf row for out-of-range
GRID = 2048                  # global graph grid rows (>= G, 16 tiles of 128)
GSH = GRID // NCORES         # 256 graphs per core after ReduceScatter

F32 = mybir.dt.float32
BF16 = mybir.dt.bfloat16
I16 = mybir.dt.int16

_cache = {}


# ---------------------------------------------------------------- host prep
def _preprocess(edge_index, batch):
    src = np.concatenate([edge_index[0], np.arange(N, dtype=np.int64)]).astype(np.int64)
    dst = np.concatenate([edge_index[1], np.arange(N, dtype=np.int64)]).astype(np.int64)
    core = dst // NSH
    dloc = (dst - core * NSH).astype(np.int64)
    blk = dloc // 128
    # padded-global table row of the source node (core-major, stride NSHP)
    spad = (src // NSH) * NSHP + (src % NSH)
    tab = (spad >= HALF).astype(np.int64)

    # group key per edge: (core, blk, tab)
    key = (core * NBLK + blk) * 2 + tab
    order = np.argsort(key, kind="stable")
    src_s, dst_s, key_s = spad[order], dst[order], key[order]
    counts = np.bincount(key_s, minlength=NCORES * NBLK * 2).reshape(NCORES, NBLK * 2)

    # uniform chunk counts across cores
    K = np.ceil(counts.max(axis=0) / 128.0).astype(np.int64)  # [NBLK*2]
    TOTCH = int(K.sum())
    choff = np.concatenate([[0], np.cumsum(K)])  # chunk offset per group

    # per-core flat edge slot arrays [TOTCH*128]
    srci = np.zeros((NCORES, TOTCH * 128), np.int16)
    dsti = np.zeros((NCORES, TOTCH * 128), np.int16)
    dstl = np.full((NCORES, TOTCH * 128), -1.0, np.float32)

    gstart = np.concatenate([[0], np.cumsum(counts.reshape(-1))[:-1]])
    gs = gstart.reshape(NCORES, NBLK * 2)
    for c in range(NCORES):
        for g in range(NBLK * 2):
            n = counts[c, g]
            if n == 0:
                continue
            s0 = gs[c, g]
            es, ed = src_s[s0:s0 + n], dst_s[s0:s0 + n]
            o0 = choff[g] * 128
            t = g & 1
            srci[c, o0:o0 + n] = (es - t * HALF).astype(np.int16)
            dl = (ed - c * NSH).astype(np.int64)
            dsti[c, o0:o0 + n] = dl.astype(np.int16)
            dstl[c, o0:o0 + n] = (dl - (g // 2) * 128).astype(np.float32)

    # gather runs: per group, runs of <=8 chunks
    gathers = []  # (tab, chunk0, nch)
    for g in range(NBLK * 2):
        k = int(K[g])
        c0 = int(choff[g])
        while k > 0:
            nch = min(k, 8)
            gathers.append((g & 1, c0, nch))
            c0 += nch
            k -= nch

    def wrap16(v):  # [n] -> [128, n//16] column-major wrap, replicated
        n = v.shape[0]
        return np.tile(v.reshape(n // 16, 16).T, (8, 1)).astype(np.int16)

    idx_src = [np.concatenate(
        [wrap16(srci[c, c0 * 128:(c0 + nch) * 128]) for (_, c0, nch) in gathers], axis=1)
        for c in range(NCORES)]
    idx_dst = [np.concatenate(
        [wrap16(dsti[c, c0 * 128:(c0 + nch) * 128]) for (_, c0, nch) in gathers], axis=1)
        for c in range(NCORES)]
    dstl_t = [dstl[c].reshape(TOTCH, 128).T.copy() for c in range(NCORES)]

    # block boundaries in chunk space: block b covers chunks [choff[2b], choff[2b+2])
    blk_first = [int(choff[2 * b]) for b in range(NBLK)]
    blk_last = [int(choff[2 * b + 2]) - 1 for b in range(NBLK)]

    # graph-local window values per (lane, block, window)
    g0 = [int(batch[c * NSH]) for c in range(NCORES)]
    glw = []
    for c in range(NCORES):
        gmax = int(batch[(c + 1) * NSH - 1]) if c < NCORES - 1 else int(batch[N - 1])
        assert gmax - g0[c] < NWIN * 128, "graph-local id range exceeds windows"
        a = np.full((128, NBLK * NWIN), -1.0, np.float32)
        for b in range(NBLK):
            nn = min(128, NSH - b * 128)
            nodes = c * NSH + b * 128 + np.arange(nn)
            gl = batch[nodes] - g0[c]
            for w in range(NWIN):
                a[:nn, b * NWIN + w] = gl - 128 * w
        glw.append(a)

    # grid gather indices: grid row r <- winbuf row (r - g0) if in window
    # range else the zeroed row ZROW
    gridx = []
    for c in range(NCORES):
        r = np.arange(GRID, dtype=np.int64) - g0[c]
        idx = np.where((r >= 0) & (r < NWIN * 128), r, ZROW).astype(np.int16)
        gridx.append(np.concatenate(
            [np.tile(idx[t * 128:(t + 1) * 128].reshape(8, 16).T, (8, 1))
             for t in range(GRID // 128)], axis=1))

    return dict(TOTCH=TOTCH, gathers=gathers, blk_first=blk_first, blk_last=blk_last,
                idx_src=idx_src, idx_dst=idx_dst, dstl=dstl_t, glw=glw, g0=g0,
                gridx=gridx)


def _wcat(Wmat, a_vec):
    # [fin, H*F] weight + per-head attention vec -> [fin, H] alpha weight
    fin = Wmat.shape[0]
    Wr = Wmat.reshape(fin, H, F)
    return np.einsum("fhk,hk->fh", Wr, a_vec)


def _fmajor(Wmat):
    # [fin, H*F] h-major columns -> f-major columns [fin, F*H]
    fin = Wmat.shape[0]
    return Wmat.reshape(fin, H, F).transpose(0, 2, 1).reshape(fin, H * F)


# ------------------------------------------------------------- device build
def _edge_pass(nc, tc, ctx, meta, tabA, tabB, adtab, tagb, consts, epil):
    """Shared edge-processing pass. epil(b, num_ps, den_ps) emits the block
    epilogue after the block's last chunk. f-major head layout throughout."""
    IC_off = 0
    sb = ctx.enter_context(tc.tile_pool(name=f"eg{tagb}", bufs=3))
    sbm = ctx.enter_context(tc.tile_pool(name=f"em{tagb}", bufs=6))
    psN = ctx.enter_context(tc.tile_pool(name=f"pn{tagb}", bufs=2, space="PSUM"))
    psD = ctx.enter_context(tc.tile_pool(name=f"pd{tagb}", bufs=2, space="PSUM"))

    iota_bf = consts["iota_bf"]
    dstl_sb = consts["dstl_sb"]
    isrc_sb = consts["isrc_sb"]
    idst_sb = consts["idst_sb"]

    num_ps = den_ps = None
    cur_blk = -1
    for (t, c0, nch) in meta["gathers"]:
        n = nch * 128
        cols = nch * 8
        gt = sb.tile([128, nch, 640], BF16, tag="maing")
        nc.gpsimd.dma_gather(
            out_ap=gt[:], in_ap=(tabA if t == 0 else tabB),
            idxs_ap=isrc_sb[:, IC_off:IC_off + cols],
            num_idxs=n, num_idxs_reg=n, elem_size=640)
        adt = sb.tile([128, nch, 128], BF16, tag="adg")
        nc.gpsimd.dma_gather(
            out_ap=adt[:], in_ap=adtab[:],
            idxs_ap=idst_sb[:, IC_off:IC_off + cols],
            num_idxs=n, num_idxs_reg=n, elem_size=128)
        IC_off += cols

        # e = as[src] + ad[dst], one batched 3D-AP add per run
        e_st = sb.tile([128, nch, 8], F32, tag="est")
        nc.vector.tensor_tensor(
            out=e_st[:], in0=gt[:, :, 512:520], in1=adt[:, :, 0:8],
            op=mybir.AluOpType.add)
        e_fl = e_st[:].rearrange("p a b -> p (a b)")
        t_sc = sb.tile([128, nch * 8], F32, tag="esc")
        nc.vector.tensor_scalar(out=t_sc[:], in0=e_fl, scalar1=NEG, scalar2=None,
                                op0=mybir.AluOpType.mult)
        nc.vector.tensor_tensor(out=e_fl, in0=e_fl, in1=t_sc[:],
                                op=mybir.AluOpType.max)
        ex_st = sb.tile([128, nch * 8], BF16, tag="exs")
        nc.scalar.activation(ex_st[:], e_fl, mybir.ActivationFunctionType.Exp)

        for j in range(nch):
            ch = c0 + j
            if num_ps is None or ch > meta["blk_last"][cur_blk]:
                cur_blk += 1
                num_ps = psN.tile([128, 512], F32, tag="nps")
                den_ps = psD.tile([128, 8], F32, tag="dps")
            S = sbm.tile([128, 128], BF16, tag="S")
            nc.vector.tensor_scalar(
                out=S[:], in0=iota_bf[:], scalar1=dstl_sb[:, ch:ch + 1],
                scalar2=None, op0=mybir.AluOpType.is_equal)
            # msg[e, (f h)] = h[e, (f h)] * ex[e, h]  (2x_1P: last dim step 1)
            msg = sbm.tile([128, 512], BF16, tag="msg")
            nc.vector.tensor_tensor(
                out=msg[:].rearrange("p (f h) -> p f h", f=F),
                in0=gt[:, j, 0:512].rearrange("p (f h) -> p f h", f=F),
                in1=ex_st[:, 8 * j:8 * j + 8].unsqueeze(1).to_broadcast([128, F, H]),
                op=mybir.AluOpType.mult)
            first = ch == meta["blk_first"][cur_blk]
            last = ch == meta["blk_last"][cur_blk]
            nc.tensor.matmul(num_ps[:], lhsT=S[:], rhs=msg[:], start=first, stop=last)
            nc.tensor.matmul(den_ps[:], lhsT=S[:], rhs=ex_st[:, 8 * j:8 * j + 8],
                             start=first, stop=last)
            if last:
                epil(cur_blk, num_ps, den_ps)


def _load_edge_consts(nc, tc, ctx, meta, inp, tagb):
    consts = {}
    cp = ctx.enter_context(tc.tile_pool(name=f"ec{tagb}", bufs=1))
    IC = sum(nch * 8 for (_, _, nch) in meta["gathers"])
    isrc_sb = cp.tile([128, IC], I16)
    nc.sync.dma_start(isrc_sb[:], inp["idx_src"][:])
    idst_sb = cp.tile([128, IC], I16)
    nc.sync.dma_start(idst_sb[:], inp["idx_dst"][:])
    dstl_sb = cp.tile([128, meta["TOTCH"]], F32)
    nc.sync.dma_start(dstl_sb[:], inp["dstl"][:])
    iota_bf = cp.tile([128, 128], BF16)
    nc.sync.dma_start(iota_bf[:], inp["iota_bf"][:])
    consts.update(isrc_sb=isrc_sb, idst_sb=idst_sb, dstl_sb=dstl_sb, iota_bf=iota_bf)
    return consts


def _emit_iter(nc, tc, ctx, meta, io, groups, it):
    """Emit one full kernel iteration. io: dict of dram tensors."""
    tg = f"_{it}"
    cp = ctx.enter_context(tc.tile_pool(name=f"wc{tg}", bufs=1))
    w1_sb = cp.tile([32, 520], BF16)
    nc.sync.dma_start(w1_sb[:], io["i_w1"][:])
    wad1_sb = cp.tile([32, 8], BF16)
    nc.sync.dma_start(wad1_sb[:], io["i_wad1"][:])
    b1_sb = cp.tile([128, 64], F32)
    nc.sync.dma_start(b1_sb[:], io["i_b1"][:])
    w2_sb = cp.tile([64, 520], BF16)
    nc.sync.dma_start(w2_sb[:], io["i_w2"][:])
    wad2_sb = cp.tile([64, 8], BF16)
    nc.sync.dma_start(wad2_sb[:], io["i_wad2"][:])
    b2_sb = cp.tile([128, 512], F32)
    nc.sync.dma_start(b2_sb[:], io["i_b2"][:])
    glw_sb = cp.tile([128, NBLK * NWIN], F32)
    nc.sync.dma_start(glw_sb[:], io["i_glw"][:])
    gridx_sb = cp.tile([128, GRID // 16], I16)
    nc.sync.dma_start(gridx_sb[:], io["i_gridx"][:])
    ident_bf = cp.tile([128, 128], BF16)
    make_identity(nc, ident_bf[:])
    xo_sb = cp.tile([32, NSHP], BF16)
    nc.sync.dma_start(xo_sb[:], io["i_xTown"][:])
    h1own = cp.tile([128, NBLK * 64], BF16)
    consts = _load_edge_consts(nc, tc, ctx, meta, dict(
        idx_src=io["i_isrc"], idx_dst=io["i_idst"], dstl=io["i_dstl"],
        iota_bf=io["i_iota"]), tg)

    tab1, adtab1 = io["tab1"], io["adtab1"]
    tab2, adtab2 = io["tab2"], io["adtab2"]
    h1locT, h1Tfull = io["h1locT"], io["h1Tfull"]
    winbuf, grid, gmine = io["winbuf"], io["grid"], io["gmine"]

    # ---------------- layer-1 adtab (own shard) + replicated full table
    with ExitStack() as tctx:
        psA = tctx.enter_context(tc.tile_pool(name=f"t1pa{tg}", bufs=2, space="PSUM"))
        rowp = tctx.enter_context(tc.tile_pool(name=f"t1row{tg}", bufs=3))
        for b in range(NBLK):
            dps = psA.tile([128, 8], F32, tag="dps")
            nc.tensor.matmul(dps[:], lhsT=xo_sb[:, b * 128:(b + 1) * 128],
                             rhs=wad1_sb[:, 0:8], start=True, stop=True)
            adrow = rowp.tile([128, 128], BF16, tag="adrow")
            nc.vector.memset(adrow[:, 8:128], 0.0)
            nc.vector.tensor_copy(adrow[:, 0:8], dps[:])
            nc.sync.dma_start(adtab1[b * 128:(b + 1) * 128, :], adrow[:])

    with ExitStack() as tctx:
        ps5 = tctx.enter_context(tc.tile_pool(name=f"t1p5{tg}", bufs=2, space="PSUM"))
        ps8 = tctx.enter_context(tc.tile_pool(name=f"t1p8{tg}", bufs=2, space="PSUM"))
        rowp = tctx.enter_context(tc.tile_pool(name=f"t1row{tg}b", bufs=4))
        xsp = tctx.enter_context(tc.tile_pool(name=f"t1xs{tg}", bufs=2))
        for cs in range(NCORES):
            xs = xsp.tile([32, NSHP], BF16, tag="xs")
            nc.sync.dma_start(xs[:], io["i_xT"][:, cs * NSHP:(cs + 1) * NSHP])
            for b in range(NBLK):
                lhsT = xs[:, b * 128:(b + 1) * 128]
                hps = ps5.tile([128, 512], F32, tag="hps")
                nc.tensor.matmul(hps[:], lhsT=lhsT, rhs=w1_sb[:, 0:512],
                                 start=True, stop=True)
                aps = ps8.tile([128, 8], F32, tag="aps")
                nc.tensor.matmul(aps[:], lhsT=lhsT, rhs=w1_sb[:, 512:520],
                                 start=True, stop=True)
                row = rowp.tile([128, 640], BF16, tag="row")
                if b % 2 == 0:
                    nc.scalar.copy(row[:, 0:512], hps[:])
                else:
                    nc.vector.tensor_copy(row[:, 0:512], hps[:])
                nc.vector.tensor_copy(row[:, 512:520], aps[:])
                r0 = cs * NSHP + b * 128
                nc.sync.dma_start(tab1[r0:r0 + 128, 0:520], row[:, 0:520])

    # ---------------- layer-1 edge pass -> h1own + h1locT + adtab2
    with ExitStack() as ectx:
        ep = ectx.enter_context(tc.tile_pool(name=f"ep1{tg}", bufs=3))
        psT = ectx.enter_context(tc.tile_pool(name=f"e1pt{tg}", bufs=2, space="PSUM"))
        psA = ectx.enter_context(tc.tile_pool(name=f"e1pa{tg}", bufs=2, space="PSUM"))
        htp = ectx.enter_context(tc.tile_pool(name=f"e1ht{tg}", bufs=2))

        def epil1(b, num_ps, den_ps):
            den = ep.tile([128, 8], F32, tag="den")
            nc.vector.tensor_scalar(out=den[:], in0=den_ps[:], scalar1=8.0,
                                    scalar2=1e-20, op0=mybir.AluOpType.mult,
                                    op1=mybir.AluOpType.add)
            rec = ep.tile([128, 8], F32, tag="rec")
            nc.vector.reciprocal(rec[:], den[:])
            tmp = ep.tile([128, 512], F32, tag="tmp")
            nc.vector.tensor_tensor(
                out=tmp[:].rearrange("p (f h) -> p f h", f=F),
                in0=num_ps[:].rearrange("p (f h) -> p f h", f=F),
                in1=rec[:].unsqueeze(1).to_broadcast([128, F, H]),
                op=mybir.AluOpType.mult)
            t3 = tmp[:].rearrange("p (f h) -> p f h", f=F)
            a4 = ep.tile([128, 256], F32, tag="a4")
            a4v = a4[:].rearrange("p (f h) -> p f h", f=F)
            nc.vector.tensor_tensor(
                out=a4v, in0=t3[:, :, 0:4], in1=t3[:, :, 4:8],
                op=mybir.AluOpType.add)
            a2 = ep.tile([128, 128], F32, tag="a2")
            a2v = a2[:].rearrange("p (f h) -> p f h", f=F)
            nc.vector.tensor_tensor(
                out=a2v, in0=a4v[:, :, 0:2], in1=a4v[:, :, 2:4],
                op=mybir.AluOpType.add)
            a1 = ep.tile([128, 64], F32, tag="a1")
            nc.vector.tensor_tensor(out=a1[:].unsqueeze(2), in0=a2v[:, :, 0:1],
                                    in1=a2v[:, :, 1:2], op=mybir.AluOpType.add)
            nc.vector.tensor_tensor(out=h1own[:, b * 64:(b + 1) * 64],
                                    in0=a1[:], in1=b1_sb[:],
                                    op=mybir.AluOpType.add)
            # transpose h1 block -> h1locT column block + layer-2 adtab row
            tps = psT.tile([64, 128], BF16, tag="tps")
            nc.tensor.transpose(tps[:], h1own[:, b * 64:(b + 1) * 64], ident_bf[:])
            hT = htp.tile([64, 128], BF16, tag="hT")
            nc.scalar.copy(hT[:], tps[:])
            nc.sync.dma_start(h1locT[:, b * 128:(b + 1) * 128], hT[:])
            dps = psA.tile([128, 8], F32, tag="dps")
            nc.tensor.matmul(dps[:], lhsT=hT[:], rhs=wad2_sb[:, 0:8],
                             start=True, stop=True)
            adrow = ep.tile([128, 128], BF16, tag="adrow")
            nc.vector.memset(adrow[:, 8:128], 0.0)
            nc.vector.tensor_copy(adrow[:, 0:8], dps[:])
            nc.sync.dma_start(adtab2[b * 128:(b + 1) * 128, :], adrow[:])

        _edge_pass(nc, tc, ectx, meta, tab1[0:HALF, :],
                   tab1[HALF:NPADN, :], adtab1, f"1{tg}", consts, epil1)

    # ---------------- AllGather h1^T (bf16, 0.8MB per core)
    nc.gpsimd.collective_compute(
        "AllGather", mybir.AluOpType.bypass, replica_groups=groups,
        ins=[h1locT[:].opt()], outs=[h1Tfull[:].opt()])

    # ---------------- layer-2 replicated full table
    with ExitStack() as tctx:
        ps5 = tctx.enter_context(tc.tile_pool(name=f"t2p5{tg}", bufs=2, space="PSUM"))
        ps8 = tctx.enter_context(tc.tile_pool(name=f"t2p8{tg}", bufs=2, space="PSUM"))
        rowp = tctx.enter_context(tc.tile_pool(name=f"t2row{tg}", bufs=4))
        hsp = tctx.enter_context(tc.tile_pool(name=f"t2hs{tg}", bufs=2))
        for cs in range(NCORES):
            hs = hsp.tile([64, NSHP], BF16, tag="hs")
            nc.sync.dma_start(hs[:], h1Tfull[cs * 64:(cs + 1) * 64, :])
            for b in range(NBLK):
                lhsT = hs[:, b * 128:(b + 1) * 128]
                hps = ps5.tile([128, 512], F32, tag="hps")
                nc.tensor.matmul(hps[:], lhsT=lhsT, rhs=w2_sb[:, 0:512],
                                 start=True, stop=True)
                aps = ps8.tile([128, 8], F32, tag="aps")
                nc.tensor.matmul(aps[:], lhsT=lhsT, rhs=w2_sb[:, 512:520],
                                 start=True, stop=True)
                row = rowp.tile([128, 640], BF16, tag="row")
                if b % 2 == 0:
                    nc.scalar.copy(row[:, 0:512], hps[:])
                else:
                    nc.vector.tensor_copy(row[:, 0:512], hps[:])
                nc.vector.tensor_copy(row[:, 512:520], aps[:])
                r0 = cs * NSHP + b * 128
                nc.sync.dma_start(tab2[r0:r0 + 128, 0:520], row[:, 0:520])

    # ---------------- layer-2 edge pass + window pooling
    with ExitStack() as ectx:
        ep = ectx.enter_context(tc.tile_pool(name=f"ep2{tg}", bufs=3))
        sgp = ectx.enter_context(tc.tile_pool(name=f"sg{tg}", bufs=3))
        psG = ectx.enter_context(tc.tile_pool(name=f"psg{tg}", bufs=1, space="PSUM"))
        gw_ps = []
        for w in range(NWIN):
            gw_tile = psG.tile([128, 512], F32, tag=f"gw{w}")
            gw_ps.append(gw_tile)

        def epil2(b, num_ps, den_ps):
            den = ep.tile([128, 8], F32, tag="den")
            nc.vector.tensor_scalar(out=den[:], in0=den_ps[:], scalar1=1e-20,
                                    scalar2=None, op0=mybir.AluOpType.add)
            rec = ep.tile([128, 8], F32, tag="rec")
            nc.vector.reciprocal(rec[:], den[:])
            o2f = ep.tile([128, 512], F32, tag="o2f")
            nc.vector.tensor_tensor(
                out=o2f[:].rearrange("p (f h) -> p f h", f=F),
                in0=num_ps[:].rearrange("p (f h) -> p f h", f=F),
                in1=rec[:].unsqueeze(1).to_broadcast([128, F, H]),
                op=mybir.AluOpType.mult)
            o2 = ep.tile([128, 512], BF16, tag="o2")
            nc.vector.tensor_tensor(out=o2[:], in0=o2f[:], in1=b2_sb[:],
                                    op=mybir.AluOpType.add)
            for w in range(NWIN):
                Sg = sgp.tile([128, 128], BF16, tag="Sg")
                nc.vector.tensor_scalar(
                    out=Sg[:], in0=consts["iota_bf"][:],
                    scalar1=glw_sb[:, b * NWIN + w:b * NWIN + w + 1],
                    scalar2=None, op0=mybir.AluOpType.is_equal)
                nc.tensor.matmul(gw_ps[w][:], lhsT=Sg[:], rhs=o2[:],
                                 start=(b == 0), stop=(b == NBLK - 1))

        _edge_pass(nc, tc, ectx, meta, tab2[0:HALF, :],
                   tab2[HALF:NPADN, :], adtab2, f"2{tg}", consts, epil2)

        # windows -> winbuf rows [0, 384); zero rows [384, 512)
        zt = ep.tile([128, 512], F32, tag="zt")
        nc.gpsimd.memset(zt[:], 0.0)
        nc.sync.dma_start(winbuf[NWIN * 128:WROWS, :], zt[:])
        for w in range(NWIN):
            wsb = ep.tile([128, 512], F32, tag="wsb")
            nc.vector.tensor_copy(wsb[:], gw_ps[w][:])
            nc.sync.dma_start(winbuf[w * 128:(w + 1) * 128, :], wsb[:])

    # ---------------- grid assembly + ReduceScatter
    with ExitStack() as gctx:
        gp = gctx.enter_context(tc.tile_pool(name=f"gridp{tg}", bufs=3))
        for t in range(GRID // 128):
            gtile = gp.tile([128, 1, 512], F32, tag="gtile")
            nc.gpsimd.dma_gather(
                out_ap=gtile[:], in_ap=winbuf[:],
                idxs_ap=gridx_sb[:, t * 8:(t + 1) * 8],
                num_idxs=128, num_idxs_reg=128, elem_size=512)
            nc.sync.dma_start(grid[t * 128:(t + 1) * 128, :],
                              gtile[:, 0, :])

    nc.gpsimd.collective_compute(
        "ReduceScatter", mybir.AluOpType.add, replica_groups=groups,
        ins=[grid[:].opt()], outs=[gmine[:].opt()])

    # ---------------- MLP on own 256 graphs
    with ExitStack() as mctx:
        mw = mctx.enter_context(tc.tile_pool(name=f"mw{tg}", bufs=1))
        fw1, fw2 = [], []
        for k in range(4):
            fw1_t = mw.tile([128, 512], BF16, tag=f"fw1{k}")
            fw1.append(fw1_t)
            fw2_t = mw.tile([128, 512], BF16, tag=f"fw2{k}")
            fw2.append(fw2_t)
        for k in range(4):
            nc.sync.dma_start(fw1[k][:], io["i_fw1"][k * 128:(k + 1) * 128, :])
            nc.sync.dma_start(fw2[k][:], io["i_fw2"][k * 128:(k + 1) * 128, :])
        fw3 = mw.tile([128, 4], BF16)
        nc.sync.dma_start(fw3[:], io["i_fw3"][:])
        fb1 = mw.tile([128, 4], F32)
        nc.sync.dma_start(fb1[:], io["i_fb1"][:])
        fb2 = mw.tile([128, 4], F32)
        nc.sync.dma_start(fb2[:], io["i_fb2"][:])
        fb3 = mw.tile([1, 1], F32)
        nc.sync.dma_start(fb3[:], io["i_fb3"][:])
        ident_f = mw.tile([128, 128], F32)
        make_identity(nc, ident_f[:])

        gp = mctx.enter_context(tc.tile_pool(name=f"mg{tg}", bufs=2))
        psT = mctx.enter_context(tc.tile_pool(name=f"mpt{tg}", bufs=2, space="PSUM"))
        psA = mctx.enter_context(tc.tile_pool(name=f"mpa{tg}", bufs=2, space="PSUM"))
        psO = mctx.enter_context(tc.tile_pool(name=f"mpo{tg}", bufs=2, space="PSUM"))
        ap_ = mctx.enter_context(tc.tile_pool(name=f"ma{tg}", bufs=2))

        for gt in range(GSH // 128):
            gl = gp.tile([128, 512], F32, tag="gl")
            nc.sync.dma_start(gl[:], gmine[gt * 128:(gt + 1) * 128, :])
            gTs = []
            for k in range(4):
                tps = psT.tile([128, 128], F32, tag="tps")
                nc.tensor.transpose(tps[:], gl[:, k * 128:(k + 1) * 128],
                                    ident_f[:])
                gT = ap_.tile([128, 128], BF16, tag=f"gT{k}")
                nc.vector.tensor_copy(gT[:], tps[:])
                gTs.append(gT)
            a1s, a2s = [], []
            for m in range(4):
                aps = psA.tile([128, 128], F32, tag="aps")
                for k in range(4):
                    nc.tensor.matmul(aps[:], lhsT=fw1[k][:, m * 128:(m + 1) * 128],
                                     rhs=gTs[k][:], start=(k == 0), stop=(k == 3))
                a1 = ap_.tile([128, 128], BF16, tag=f"a1{m}")
                nc.scalar.activation(a1[:], aps[:],
                                     mybir.ActivationFunctionType.Relu,
                                     bias=fb1[:, m:m + 1])
                a1s.append(a1)
            for m in range(4):
                aps = psA.tile([128, 128], F32, tag="bps")
                for k in range(4):
                    nc.tensor.matmul(aps[:], lhsT=fw2[k][:, m * 128:(m + 1) * 128],
                                     rhs=a1s[k][:], start=(k == 0), stop=(k == 3))
                a2 = ap_.tile([128, 128], BF16, tag=f"a2{m}")
                nc.scalar.activation(a2[:], aps[:],
                                     mybir.ActivationFunctionType.Relu,
                                     bias=fb2[:, m:m + 1])
                a2s.append(a2)
            ops = psO.tile([128, 128], F32, tag="ops")
            for k in range(4):
                nc.tensor.matmul(ops[0:1, :], lhsT=fw3[:, k:k + 1], rhs=a2s[k][:],
                                 start=(k == 0), stop=(k == 3))
            osb = ap_.tile([128, 128], F32, tag="osb")
            nc.scalar.activation(osb[0:1, :], ops[0:1, :],
                                 mybir.ActivationFunctionType.Identity,
                                 bias=fb3[0:1, 0:1])
            nc.sync.dma_start(io["o_out"][0:1, gt * 128:(gt + 1) * 128], osb[0:1, :])


def _build_merged(meta, iters=1):
    nc = bacc.Bacc("TRN2", target_bir_lowering=False, debug=False, num_devices=NCORES)
    IC = sum(nch * 8 for (_, _, nch) in meta["gathers"])
    groups = [list(range(NCORES))]

    io = {}
    io["i_xT"] = nc.dram_tensor("xT", [32, NPADN], BF16, kind="ExternalInput")
    io["i_xTown"] = nc.dram_tensor("xTown", [32, NSHP], BF16, kind="ExternalInput")
    io["i_w1"] = nc.dram_tensor("w1cat", [32, 520], BF16, kind="ExternalInput")
    io["i_wad1"] = nc.dram_tensor("wad1", [32, 8], BF16, kind="ExternalInput")
    io["i_b1"] = nc.dram_tensor("b1rep", [128, 64], F32, kind="ExternalInput")
    io["i_w2"] = nc.dram_tensor("w2cat", [64, 520], BF16, kind="ExternalInput")
    io["i_wad2"] = nc.dram_tensor("wad2", [64, 8], BF16, kind="ExternalInput")
    io["i_b2"] = nc.dram_tensor("b2rep", [128, 512], F32, kind="ExternalInput")
    io["i_isrc"] = nc.dram_tensor("idx_src", [128, IC], I16, kind="ExternalInput")
    io["i_idst"] = nc.dram_tensor("idx_dst", [128, IC], I16, kind="ExternalInput")
    io["i_dstl"] = nc.dram_tensor("dstl", [128, meta["TOTCH"]], F32, kind="ExternalInput")
    io["i_iota"] = nc.dram_tensor("iota_bf", [128, 128], BF16, kind="ExternalInput")
    io["i_glw"] = nc.dram_tensor("glw", [128, NBLK * NWIN], F32, kind="ExternalInput")
    io["i_gridx"] = nc.dram_tensor("gridx", [128, GRID // 16], I16, kind="ExternalInput")
    io["i_fw1"] = nc.dram_tensor("fcw1", [512, 512], BF16, kind="ExternalInput")
    io["i_fw2"] = nc.dram_tensor("fcw2", [512, 512], BF16, kind="ExternalInput")
    io["i_fw3"] = nc.dram_tensor("fcw3", [128, 4], BF16, kind="ExternalInput")
    io["i_fb1"] = nc.dram_tensor("fcb1", [128, 4], F32, kind="ExternalInput")
    io["i_fb2"] = nc.dram_tensor("fcb2", [128, 4], F32, kind="ExternalInput")
    io["i_fb3"] = nc.dram_tensor("fcb3", [1, 1], F32, kind="ExternalInput")
    io["o_out"] = nc.dram_tensor("out", [1, GSH], F32, kind="ExternalOutput")

    with tile.TileContext(nc, num_cores=NCORES) as tc:
        with ExitStack() as ctx:
            dram = ctx.enter_context(tc.tile_pool(name="dram", bufs=1, space="DRAM"))
            io["tab1"] = dram.tile([NPADN, 640], BF16, name="tab1")
            io["adtab1"] = dram.tile([NSHP, 128], BF16, name="adtab1")
            io["tab2"] = dram.tile([NPADN, 640], BF16, name="tab2")
            io["adtab2"] = dram.tile([NSHP, 128], BF16, name="adtab2")
            io["h1locT"] = dram.tile([64, NSHP], BF16, name="h1locT")
            io["winbuf"] = dram.tile([WROWS, 512], F32, name="winbuf")
            io["grid"] = dram.tile([GRID, 512], F32, name="grid")

            for it in range(iters):
                # collective outputs: single-writer rule -> one tile per iter
                io2 = dict(io)
                io2["h1Tfull"] = dram.tile(
                    [NCORES * 64, NSHP], BF16, addr_space="Shared",
                    name=f"h1Tfull{it}")
                io2["gmine"] = dram.tile([GSH, 512], F32, name=f"gmine{it}")
                with ExitStack() as bctx:
                    _emit_iter(nc, tc, bctx, meta, io2, groups, it)

    nc.compile()
    return nc


# ------------------------------------------------- cached PJRT runner
def _ensure_runner(nc, key):
    """Build the jitted shard_map executor for nc (once per program)."""
    import jax
    from jax.sharding import Mesh, PartitionSpec, NamedSharding
    from jax.experimental.shard_map import shard_map
    from concourse import bass2jax
    from concourse.bass2jax import _bass_exec_p, partition_id_tensor

    st = _cache.setdefault(key, {})
    if "fn" in st:
        return st
    bass2jax.install_neuronx_cc_hook()
    partition_name = (nc.partition_id_tensor.name
                      if nc.partition_id_tensor else None)
    in_names, out_names, out_avals = [], [], []
    for alloc in nc.m.functions[0].allocations:
        if not isinstance(alloc, mybir.MemoryLocationSet):
            continue
        name = alloc.memorylocations[0].name
        if alloc.kind == "ExternalInput":
            if name != partition_name:
                in_names.append(name)
        elif alloc.kind == "ExternalOutput":
            shape = tuple(alloc.tensor_shape)
            dtype = mybir.dt.np(alloc.dtype)
            out_names.append(name)
            out_avals.append(jax.core.ShapedArray(shape, dtype))
    n_params = len(in_names)
    all_names = list(in_names) + list(out_names)
    if partition_name is not None:
        all_names.append(partition_name)
    donate = tuple(range(n_params, n_params + len(out_names)))

    def _body(*args):
        operands = list(args)
        if partition_name is not None:
            operands.append(partition_id_tensor())
        outs = _bass_exec_p.bind(
            *operands, out_avals=tuple(out_avals), in_names=tuple(all_names),
            out_names=tuple(out_names), lowering_input_output_aliases=(),
            sim_require_finite=True, sim_require_nnan=True, nc=nc)
        return tuple(outs)

    devices = jax.devices()[:NCORES]
    mesh = Mesh(np.asarray(devices), ("core",))
    spec_in = (PartitionSpec("core"),) * (n_params + len(out_names))
    spec_out = (PartitionSpec("core"),) * len(out_names)
    fn = jax.jit(shard_map(_body, mesh=mesh, in_specs=spec_in,
                           out_specs=spec_out, check_rep=False),
                 donate_argnums=donate, keep_unused=True)
    st.update(fn=fn, in_names=in_names, out_names=out_names,
              out_avals=out_avals,
              shard=NamedSharding(mesh, PartitionSpec("core")))
    return st


def _host_prep(raws, meta):
    (x, edge_index, batch, W1, a_src1, a_dst1, b1, W2, a_src2, a_dst2,
     b2, fcW1, fcb1, fcW2, fcb2, fcW3, fcb3) = raws
    x = np.asarray(x, np.float32)
    xpad = np.zeros((NPADN, NF), np.float32)
    for c in range(NCORES):
        xpad[c * NSHP:c * NSHP + NSH] = x[c * NSH:(c + 1) * NSH]
    xT = np.ascontiguousarray(xpad.T).astype(BF)
    W1f = np.asarray(W1, np.float32)
    w1cat = np.concatenate(
        [_fmajor(W1f), _wcat(W1f, np.asarray(a_src1, np.float32))],
        axis=1).astype(BF)
    wad1 = _wcat(W1f, np.asarray(a_dst1, np.float32)).astype(BF)
    W2f = np.asarray(W2, np.float32)
    w2cat = np.concatenate(
        [_fmajor(W2f), _wcat(W2f, np.asarray(a_src2, np.float32))],
        axis=1).astype(BF)
    wad2 = _wcat(W2f, np.asarray(a_dst2, np.float32)).astype(BF)
    b1rep = np.tile(np.asarray(b1, np.float32)[None, :], (128, 1))
    b2f = np.asarray(b2, np.float32).reshape(H, F).T.reshape(HF)  # f-major
    b2rep = np.tile(b2f[None, :], (128, 1))
    iota_bf = np.tile(np.arange(128, dtype=np.float32), (128, 1)).astype(BF)
    # fcW1 rows permuted to f-major g layout
    fcW1p = np.asarray(fcW1, np.float32).reshape(H, F, 512).transpose(1, 0, 2)\
        .reshape(HF, 512)
    fcb1a = np.asarray(fcb1, np.float32).reshape(4, 128).T.copy()
    fcb2a = np.asarray(fcb2, np.float32).reshape(4, 128).T.copy()
    fw3a = np.asarray(fcW3, np.float32).reshape(4, 128).T.astype(BF).copy()

    in_maps = []
    for c in range(NCORES):
        xTown = np.ascontiguousarray(
            xpad[c * NSHP:(c + 1) * NSHP].T).astype(BF)
        in_maps.append(dict(
            xT=xT, xTown=xTown, w1cat=w1cat, wad1=wad1, b1rep=b1rep,
            w2cat=w2cat, wad2=wad2, b2rep=b2rep,
            idx_src=meta["idx_src"][c], idx_dst=meta["idx_dst"][c],
            dstl=meta["dstl"][c], iota_bf=iota_bf, glw=meta["glw"][c],
            gridx=meta["gridx"][c],
            fcw1=fcW1p.astype(BF),
            fcw2=np.asarray(fcW2, np.float32).astype(BF), fcw3=fw3a,
            fcb1=fcb1a, fcb2=fcb2a,
            fcb3=np.asarray(fcb3, np.float32).reshape(1, 1)))
    return in_maps


# ----------------------------------------------------------------- kernel()
def kernel(x, edge_index, batch, W1, a_src1, a_dst1, b1, W2, a_src2, a_dst2, b2,
           fcW1, fcb1, fcW2, fcb2, fcW3, fcb3):
    import jax

    raws = (x, edge_index, batch, W1, a_src1, a_dst1, b1, W2, a_src2, a_dst2,
            b2, fcW1, fcb1, fcW2, fcb2, fcW3, fcb3)
    raws = tuple(np.asarray(a) for a in raws)
    cached = _cache.get("raws")
    same = [cached is not None and len(cached) == len(raws)
            and a.shape == b.shape and np.array_equal(a, b)
            for a, b in zip(raws, cached or raws)]
    hit = bool(same) and all(same)

    if not hit:
        graph_same = bool(same) and same[1] and same[2] and "meta" in _cache
        _cache["raws"] = tuple(np.array(a, copy=True) for a in raws)
        meta = _cache["meta"] if graph_same else _preprocess(
            np.asarray(raws[1]), np.asarray(raws[2]))
        _cache["meta"] = meta
        key = (meta["TOTCH"], len(meta["gathers"]), tuple(meta["g0"]))
        if _cache.get("progkey") != key:
            _cache["prog"] = _build_merged(meta)
            _cache["progkey"] = key
            _cache.pop("runner", None)
            _cache.pop("benchrunner", None)
            _cache.pop("benchprog", None)

        in_maps = _host_prep(raws, meta)
        _cache["in_maps"] = in_maps
        st = _ensure_runner(_cache["prog"], "runner")
        st["dev_args"] = [
            jax.device_put(
                np.concatenate([np.asarray(m[name]) for m in in_maps], axis=0),
                st["shard"])
            for name in st["in_names"]]

    st = _cache["runner"]
    zeros = [jax.device_put(
        np.zeros((NCORES * av.shape[0], *av.shape[1:]), av.dtype), st["shard"])
        for av in st["out_avals"]]

    t0 = time.time()
    outs = st["fn"](*st["dev_args"], *zeros)
    res = [np.asarray(o) for o in outs]
    LAST_TIMES.clear()
    LAST_TIMES["p"] = time.time() - t0

    oi = st["out_names"].index("out")
    out = res[oi].reshape(NCORES, GSH).reshape(-1)  # [2048] in core order
    return out[:G].astype(np.float32).reshape(G, 1)


# ----------------------------------------------------------- bench (K iters)
BENCH_ITERS = 16


def bench_call():
    """Run the K-iteration program once; returns (wall_s, out[G,1]).

    Requires kernel() to have been called at least once (device inputs
    cached). The K-iteration NEFF executes the full kernel K times
    back-to-back on device, so wall = RTT + K * t_exec.
    """
    import jax
    if "benchprog" not in _cache:
        _cache["benchprog"] = _build_merged(_cache["meta"], iters=BENCH_ITERS)
    st = _ensure_runner(_cache["benchprog"], "benchrunner")
    if "dev_args" not in st:
        in_maps = _cache["in_maps"]
        st["dev_args"] = [
            jax.device_put(
                np.concatenate([np.asarray(m[name]) for m in in_maps], axis=0),
                st["shard"])
            for name in st["in_names"]]
    zeros = [jax.device_put(
        np.zeros((NCORES * av.shape[0], *av.shape[1:]), av.dtype), st["shard"])
        for av in st["out_avals"]]
    t0 = time.time()
    outs = st["fn"](*st["dev_args"], *zeros)
    res = [np.asarray(o) for o in outs]
    wall = time.time() - t0
    oi = st["out_names"].index("out")
    out = res[oi].reshape(NCORES, GSH).reshape(-1)
    return wall, out[:G].astype(np.float32).reshape(G, 1)
